# revision 1
# baseline (speedup 1.0000x reference)
"""DeepseekV3-style MoE block on 8 Trainium2 NeuronCores (expert-parallel).

Sharding strategy:
  - The 64 routed experts are sharded 8-per-core.  Expert columns are
    *rotated* per core so every core's own experts sit at columns 0..7 of its
    (rotated) router output; group-limited routing is invariant under group
    rotation because groups are scored independently.
  - The router (gate matmul + sigmoid + group top-k) is replicated.
  - The shared expert is TP-sharded on its intermediate dim (96 rows/core).
  - Each core computes a partial dense [T, H] output (shared slice + its 8
    experts' combine-weighted FFN outputs scattered back by token id), then a
    ReduceScatter(add) sums across cores; the host concatenates the 8 shards.

Token dispatch is fully on-device: dense combine weights -> per-expert
selected-token lists via gpsimd sparse_gather (compaction) -> dma_gather of
x rows per expert (capacity C=256 slots) -> gated-MLP in float32r matmuls ->
scale rows by gathered combine weights -> dma_scatter_add back into the
accumulator.
"""

import numpy as np

import concourse.bass as bass
import concourse.bacc as bacc
import concourse.mybir as mybir
import concourse.tile as tile
from concourse.bass import IndirectOffsetOnAxis  # noqa: F401  (kept for debug)
from concourse.bass_utils import run_bass_kernel_spmd
from concourse.masks import make_identity

F32 = mybir.dt.float32
F32R = mybir.dt.float32r
I16 = mybir.dt.int16
I32 = mybir.dt.int32
U32 = mybir.dt.uint32
U8 = mybir.dt.uint8

# Model constants (hardcoded per contest rules)
E = 64          # experts
TOPK = 8
NG = 8          # groups
TOPKG = 4       # groups selected
SCALE = 2.5
H = 768         # hidden
I = 384         # routed expert intermediate
SI = 768        # shared expert intermediate
T = 1024        # tokens
NCORES = 8
EPC = E // NCORES     # experts per core = 8
SIPC = SI // NCORES   # shared-intermediate rows per core = 96
C = 256               # per-expert token capacity (avg load is 128)
TCH = T // 128        # token chunks = 8
HCH = H // 128        # hidden chunks = 6
ICH = I // 128        # intermediate chunks = 3
BIG = 1.0e30


def r32(ap):
    return ap.bitcast(F32R)


def build_nc(debug=False):
    nc = bacc.Bacc(num_devices=NCORES)

    # ---------------- I/O ----------------
    xT_d = nc.declare_dram_parameter("xT", [H, T], F32, isOutput=False)
    x_d = nc.declare_dram_parameter("x_pad", [T + 1, H], F32, isOutput=False)
    gwT_d = nc.declare_dram_parameter("gwT", [H, E], F32, isOutput=False)
    eb_d = nc.declare_dram_parameter("ebias_b", [128, E], F32, isOutput=False)
    w13_d = nc.declare_dram_parameter("w13T", [EPC, H, 2 * I], F32R, isOutput=False)
    w2_d = nc.declare_dram_parameter("w2T", [EPC, I, H], F32R, isOutput=False)
    wsg_d = nc.declare_dram_parameter("wsgT", [H, SIPC], F32R, isOutput=False)
    wsu_d = nc.declare_dram_parameter("wsuT", [H, SIPC], F32R, isOutput=False)
    wsd_d = nc.declare_dram_parameter("wsdT", [SIPC, H], F32R, isOutput=False)
    tok_d = nc.declare_dram_parameter("tokid1", [T, 1], F32, isOutput=False)
    slot_d = nc.declare_dram_parameter("slotidx", [16, 16], F32, isOutput=False)
    out_d = nc.declare_dram_parameter("out", [T // NCORES, H], F32, isOutput=True)

    # ---------------- internal DRAM ----------------
    cmb_d = nc.dram_tensor("cmb_d", [T + 1, E], F32)
    vals_d = nc.dram_tensor("vals_d", [EPC, T], F32)
    idx_d = nc.dram_tensor("idx_d", [16, 8 * 16], I16)
    acc_d = nc.dram_tensor("acc_d", [T + 1, H], F32)
    rs_d = nc.dram_tensor("rs_d", [T // NCORES, H], F32)
    if debug:
        dbg_cnt_d = nc.dram_tensor("dbg_cnt_d", [1, EPC], F32)
        dbg_cntb_d = nc.dram_tensor("dbg_cntb_d", [16, EPC], F32)
        dbg_idxf_d = nc.dram_tensor("dbg_idxf_d", [16, EPC * 16], F32)

    with tile.TileContext(nc) as tc:
        with (
            tc.tile_pool(name="const", bufs=1) as constp,
            tc.tile_pool(name="xt", bufs=1) as xtp,
            tc.tile_pool(name="wts", bufs=2) as wtsp,
            tc.tile_pool(name="route", bufs=1) as routep,
            tc.tile_pool(name="small", bufs=2) as smallp,
            tc.tile_pool(name="work", bufs=2) as workp,
            tc.tile_pool(name="psum", bufs=8, space="PSUM") as psp,
        ):
            # ---------------- constants / inputs ----------------
            ident = constp.tile([128, 128], F32, tag="ident")
            make_identity(nc, ident[:])
            ebias = constp.tile([128, E], F32, tag="ebias")
            nc.sync.dma_start(out=ebias[:], in_=eb_d[:, :])
            tokid1 = constp.tile([128, TCH], F32, tag="tokid1")
            nc.sync.dma_start(
                out=tokid1[:], in_=tok_d.ap().rearrange("(c p) o -> p (c o)", p=128)
            )
            slotidx = constp.tile([16, 16], F32, tag="slotidx")
            nc.sync.dma_start(out=slotidx[:], in_=slot_d[:, :])
            ones1 = constp.tile([128, 1], F32, tag="ones1")
            nc.vector.memset(ones1[:], 1.0)

            xT = xtp.tile([128, HCH, T], F32, tag="xT")
            nc.sync.dma_start(
                out=xT[:], in_=xT_d.ap().rearrange("(k p) t -> p k t", p=128)
            )
            gwT = constp.tile([128, HCH, E], F32, tag="gwT")
            nc.sync.dma_start(
                out=gwT[:], in_=gwT_d.ap().rearrange("(k p) e -> p k e", p=128)
            )
            wsg = constp.tile([128, HCH, SIPC], F32R, tag="wsg")
            nc.sync.dma_start(
                out=wsg[:], in_=wsg_d.ap().rearrange("(k p) i -> p k i", p=128)
            )
            wsu = constp.tile([128, HCH, SIPC], F32R, tag="wsu")
            nc.sync.dma_start(
                out=wsu[:], in_=wsu_d.ap().rearrange("(k p) i -> p k i", p=128)
            )
            wsd = constp.tile([SIPC, H], F32R, tag="wsd")
            nc.sync.dma_start(out=wsd[:], in_=wsd_d[:, :])
            # f32r copy of xT for the shared-expert stage-1 rhs
            xTr = xtp.tile([128, HCH, T], F32R, tag="xTr")
            nc.vector.tensor_copy(out=xTr[:], in_=xT[:])

            # ---------------- router: logitsT = gwT.T @ xT ----------------
            lgsb = routep.tile([64, T], F32, tag="lgsb")
            for n in range(2):
                lgp = psp.tile([64, 512], F32, tag="ps")
                for k in range(HCH):
                    nc.tensor.matmul(
                        out=lgp[:],
                        lhsT=gwT[:, k, :],
                        rhs=xT[:, k, n * 512 : (n + 1) * 512],
                        start=(k == 0),
                        stop=(k == HCH - 1),
                    )
                nc.vector.tensor_copy(out=lgsb[:, n * 512 : (n + 1) * 512], in_=lgp[:])

            # routing state, per token chunk, experts on the free dim
            scores = routep.tile([128, TCH, E], F32, tag="scores")
            swb = routep.tile([128, TCH, E], F32, tag="swb")
            cmb = routep.tile([128, TCH, E], F32, tag="cmb")
            valsT = routep.tile([EPC, T], F32, tag="valsT")
            cnt_ps = psp.tile([EPC, 1], F32, tag="ps")

            for c in range(TCH):
                lt = psp.tile([128, 64], F32, tag="ps")
                nc.tensor.transpose(
                    out=lt[:], in_=lgsb[:, c * 128 : (c + 1) * 128],
                    identity=ident[:64, :64],
                )
                sc = scores[:, c, :]
                nc.scalar.activation(
                    out=sc, in_=lt[:], func=mybir.ActivationFunctionType.Sigmoid
                )
                sw = swb[:, c, :]
                nc.vector.tensor_add(out=sw, in0=sc, in1=ebias[:])

                sw3 = sw.rearrange("p (g e) -> p g e", e=NG)
                m1 = smallp.tile([128, NG], F32, tag="m1")
                nc.vector.tensor_reduce(
                    out=m1[:], in_=sw3, axis=mybir.AxisListType.X,
                    op=mybir.AluOpType.max,
                )
                eq = smallp.tile([128, NG, NG], F32, tag="eq")
                nc.vector.tensor_tensor(
                    out=eq[:], in0=sw3, in1=m1[:, :, None].to_broadcast([128, NG, NG]),
                    op=mybir.AluOpType.is_equal,
                )
                nc.vector.tensor_scalar(
                    out=eq[:], in0=eq[:], scalar1=-BIG, scalar2=None,
                    op0=mybir.AluOpType.mult,
                )
                nc.vector.tensor_add(out=eq[:], in0=eq[:], in1=sw3)
                m2 = smallp.tile([128, NG], F32, tag="m2")
                nc.vector.tensor_reduce(
                    out=m2[:], in_=eq[:], axis=mybir.AxisListType.X,
                    op=mybir.AluOpType.max,
                )
                gs = smallp.tile([128, NG], F32, tag="gs")
                nc.vector.tensor_add(out=gs[:], in0=m1[:], in1=m2[:])
                g4 = smallp.tile([128, 8], F32, tag="g4")
                nc.vector.max(out=g4[:], in_=gs[:])
                gmask = smallp.tile([128, NG], F32, tag="gmask")
                nc.vector.tensor_scalar(
                    out=gmask[:], in0=gs[:], scalar1=g4[:, TOPKG - 1 : TOPKG],
                    scalar2=None, op0=mybir.AluOpType.is_ge,
                )
                masked = smallp.tile([128, E], F32, tag="masked")
                nc.vector.tensor_tensor(
                    out=masked[:].rearrange("p (g e) -> p g e", e=NG), in0=sw3,
                    in1=gmask[:, :, None].to_broadcast([128, NG, NG]),
                    op=mybir.AluOpType.mult,
                )
                t8 = smallp.tile([128, 8], F32, tag="t8")
                nc.vector.max(out=t8[:], in_=masked[:])
                nmask = smallp.tile([128, E], F32, tag="nmask")
                nc.vector.tensor_scalar(
                    out=nmask[:], in0=masked[:], scalar1=t8[:, TOPK - 1 : TOPK],
                    scalar2=None, op0=mybir.AluOpType.is_ge,
                )
                sel = cmb[:, c, :]
                nc.vector.tensor_tensor(
                    out=sel, in0=sc, in1=nmask[:], op=mybir.AluOpType.mult
                )
                den = smallp.tile([128, 1], F32, tag="den")
                nc.vector.tensor_reduce(
                    out=den[:], in_=sel, axis=mybir.AxisListType.X,
                    op=mybir.AluOpType.add,
                )
                nc.vector.tensor_scalar(
                    out=den[:], in0=den[:], scalar1=1e-20, scalar2=None,
                    op0=mybir.AluOpType.add,
                )
                rec = smallp.tile([128, 1], F32, tag="rec")
                nc.vector.reciprocal(out=rec[:], in_=den[:])
                nc.vector.tensor_scalar(
                    out=rec[:], in0=rec[:], scalar1=SCALE, scalar2=None,
                    op0=mybir.AluOpType.mult,
                )
                nc.vector.tensor_scalar(
                    out=sel, in0=sel, scalar1=rec[:, 0:1], scalar2=None,
                    op0=mybir.AluOpType.mult,
                )
                # combine weights of this chunk -> DRAM (gather source)
                nc.sync.dma_start(out=cmb_d[c * 128 : (c + 1) * 128, :], in_=sel)

                # per-token "selected" markers for my experts (cols 0..EPC)
                mask8 = smallp.tile([128, EPC], F32, tag="mask8")
                nc.vector.tensor_scalar(
                    out=mask8[:], in0=cmb[:, c, 0:EPC], scalar1=0.0, scalar2=None,
                    op0=mybir.AluOpType.is_gt,
                )
                # counts: cnt_ps[j] += sum_t mask8[t, j]
                nc.tensor.matmul(
                    out=cnt_ps[:], lhsT=mask8[:], rhs=ones1[:],
                    start=(c == 0), stop=(c == TCH - 1),
                )
                vals = smallp.tile([128, EPC], F32, tag="vals")
                nc.vector.tensor_scalar(
                    out=vals[:], in0=mask8[:], scalar1=tokid1[:, c : c + 1],
                    scalar2=None, op0=mybir.AluOpType.mult,
                )
                nc.vector.tensor_scalar(
                    out=vals[:], in0=vals[:], scalar1=-1.0, scalar2=None,
                    op0=mybir.AluOpType.add,
                )
                vt = psp.tile([EPC, 128], F32, tag="ps")
                nc.tensor.transpose(out=vt[:], in_=vals[:], identity=ident[:])
                nc.vector.tensor_copy(
                    out=valsT[:, c * 128 : (c + 1) * 128], in_=vt[:]
                )

            # zero the dummy row of cmb_d (row T) so padded gathers get w=0
            zrow = smallp.tile([1, E], F32, tag="zrow")
            nc.vector.memset(zrow[:], 0.0)
            nc.sync.dma_start(out=cmb_d[T : T + 1, :], in_=zrow[:])

            # counts -> row layout on partition 0 (gpsimd needs base partition 0)
            cnt = routep.tile([EPC, 1], F32, tag="cnt")
            nc.vector.tensor_copy(out=cnt[:], in_=cnt_ps[:])
            cntrow_ps = psp.tile([1, EPC], F32, tag="ps")
            nc.tensor.transpose(
                out=cntrow_ps[:], in_=cnt[:], identity=ident[:EPC, :EPC]
            )
            cntrow = routep.tile([1, EPC], F32, tag="cntrow")
            nc.vector.tensor_copy(out=cntrow[:], in_=cntrow_ps[:])
            # broadcast count j to 16 partitions (for slot sanitation)
            cntb = routep.tile([16, EPC], F32, tag="cntb")
            for j in range(EPC):
                nc.gpsimd.partition_broadcast(
                    out_ap=cntb[:, j : j + 1], in_ap=cntrow[0:1, j : j + 1]
                )
            if debug:
                nc.sync.dma_start(out=dbg_cnt_d[:, :], in_=cntrow[:])
                nc.sync.dma_start(out=dbg_cntb_d[:, :], in_=cntb[:])

            # valsT -> DRAM -> per-expert 16-partition-wrapped tiles [16, 64]
            nc.sync.dma_start(out=vals_d[:, :], in_=valsT[:])
            v16 = []
            for j in range(EPC):
                vt16 = routep.tile([16, T // 16], F32, tag=f"v16_{j}")
                nc.sync.dma_start(
                    out=vt16[:],
                    in_=vals_d[j].rearrange("(p f) -> p f", p=16),
                )
                v16.append(vt16)

            # compact per-expert token lists (sparse_gather) + sanitize tails
            padT = routep.tile([16, 16], F32, tag="padT")
            nc.vector.memset(padT[:], float(T))
            idxf = routep.tile([16, EPC * 16], F32, tag="idxf")
            nfound = routep.tile([1, EPC], U32, tag="nfound")
            idxs = routep.tile([16, EPC * 16], F32, tag="idxs")
            idx16 = routep.tile([16, EPC * 16], I16, tag="idx16")
            for j in range(EPC):
                nc.gpsimd.sparse_gather(
                    out=idxf[:, j * 16 : (j + 1) * 16],
                    in_=v16[j][:],
                    num_found=nfound[:, j : j + 1],
                )
                if debug:
                    nc.sync.dma_start(
                        out=dbg_idxf_d[:, j * 16 : (j + 1) * 16],
                        in_=idxf[:, j * 16 : (j + 1) * 16],
                    )
                # sanitize: slots >= count[j] -> dummy row T (gathers zeros,
                # scatter-adds exact zeros); select() is NaN-garbage-proof.
                # NB select(out=...) must not alias on_true (it copies
                # on_false into out first).
                keep = smallp.tile([16, 16], U8, tag="keep")
                nc.vector.tensor_scalar(
                    out=keep[:], in0=slotidx[:], scalar1=cntb[:, j : j + 1],
                    scalar2=None, op0=mybir.AluOpType.is_lt,
                )
                nc.vector.select(
                    out=idxs[:, j * 16 : (j + 1) * 16], mask=keep[:],
                    on_true=idxf[:, j * 16 : (j + 1) * 16], on_false=padT[:],
                )
            nc.vector.tensor_copy(out=idx16[:], in_=idxs[:])
            # replicate idx rows to all 8 16-partition groups (via DRAM bounce;
            # step-0 AP on the DRAM side repeats the 16 rows 8x)
            nc.sync.dma_start(out=idx_d[:, :], in_=idx16[:])
            idxr = routep.tile([128, EPC * 16], I16, tag="idxr")
            nc.sync.dma_start(
                out=idxr[:],
                in_=bass.AP(idx_d, 0, [[0, 8], [EPC * 16, 16], [1, EPC * 16]]),
            )

            # ---------------- shared expert (TP slice) ----------------
            hsh = routep.tile([SIPC, T], F32R, tag="hsh")
            for n in range(2):
                hg = psp.tile([SIPC, 512], F32, tag="ps")
                hu = psp.tile([SIPC, 512], F32, tag="ps")
                for k in range(HCH):
                    nc.tensor.matmul(
                        out=hg[:], lhsT=wsg[:, k, :],
                        rhs=xTr[:, k, n * 512 : (n + 1) * 512],
                        start=(k == 0), stop=(k == HCH - 1),
                    )
                for k in range(HCH):
                    nc.tensor.matmul(
                        out=hu[:], lhsT=wsu[:, k, :],
                        rhs=xTr[:, k, n * 512 : (n + 1) * 512],
                        start=(k == 0), stop=(k == HCH - 1),
                    )
                hs_sl = hsh[:, n * 512 : (n + 1) * 512]
                nc.scalar.activation(
                    out=hs_sl, in_=hg[:], func=mybir.ActivationFunctionType.Sigmoid
                )
                nc.vector.tensor_tensor(
                    out=hs_sl, in0=hs_sl, in1=hg[:], op=mybir.AluOpType.mult
                )
                nc.vector.tensor_tensor(
                    out=hs_sl, in0=hs_sl, in1=hu[:], op=mybir.AluOpType.mult
                )
            for c in range(TCH):
                so = workp.tile([128, H], F32, tag="so")
                for n2 in range(2):
                    sp = psp.tile([128, 384], F32, tag="ps")
                    nc.tensor.matmul(
                        out=sp[:],
                        lhsT=hsh[:, c * 128 : (c + 1) * 128],
                        rhs=wsd[:, n2 * 384 : (n2 + 1) * 384],
                        start=True, stop=True,
                    )
                    nc.vector.tensor_copy(
                        out=so[:, n2 * 384 : (n2 + 1) * 384], in_=sp[:]
                    )
                nc.sync.dma_start(out=acc_d[c * 128 : (c + 1) * 128, :], in_=so[:])

            # ---------------- routed experts ----------------
            for j in range(EPC):
                w13 = wtsp.tile([128, HCH, 2 * I], F32R, tag="w13")
                nc.sync.dma_start(
                    out=w13[:],
                    in_=w13_d[j].rearrange("(k p) i -> p k i", p=128),
                )
                w2 = wtsp.tile([128, ICH, H], F32R, tag="w2")
                nc.sync.dma_start(
                    out=w2[:], in_=w2_d[j].rearrange("(k p) h -> p k h", p=128)
                )

                xg = workp.tile([128, C // 128, H], F32, tag="xg")
                nc.gpsimd.dma_gather(
                    out_ap=xg[:], in_ap=x_d[:, :],
                    idxs_ap=idxr[:, j * 16 : (j + 1) * 16],
                    num_idxs=C, num_idxs_reg=C, elem_size=H,
                )
                wg = workp.tile([128, C // 128, E], F32, tag="wg")
                nc.gpsimd.dma_gather(
                    out_ap=wg[:], in_ap=cmb_d[:, :],
                    idxs_ap=idxr[:, j * 16 : (j + 1) * 16],
                    num_idxs=C, num_idxs_reg=C, elem_size=E,
                )

                # transpose gathered tokens: xgT[h, slot]
                xgT = workp.tile([128, HCH, C], F32R, tag="xgT")
                for ci in range(C // 128):
                    for k in range(HCH):
                        tp = psp.tile([128, 128], F32, tag="ps")
                        nc.tensor.transpose(
                            out=tp[:], in_=xg[:, ci, k * 128 : (k + 1) * 128],
                            identity=ident[:],
                        )
                        nc.vector.tensor_copy(
                            out=xgT[:, k, ci * 128 : (ci + 1) * 128], in_=tp[:]
                        )

                # stage 1: h1/h3 = w1 @ xgT, w3 @ xgT   (out: [I, C] chunks)
                hj = workp.tile([128, ICH, C], F32R, tag="hj")
                for m in range(ICH):
                    h13 = psp.tile([128, 512], F32, tag="ps")
                    for k in range(HCH):
                        nc.tensor.matmul(
                            out=h13[:, 0:C],
                            lhsT=w13[:, k, m * 128 : (m + 1) * 128],
                            rhs=xgT[:, k, :],
                            start=(k == 0), stop=(k == HCH - 1),
                        )
                    for k in range(HCH):
                        nc.tensor.matmul(
                            out=h13[:, C : 2 * C],
                            lhsT=w13[:, k, I + m * 128 : I + (m + 1) * 128],
                            rhs=xgT[:, k, :],
                            start=(k == 0), stop=(k == HCH - 1),
                        )
                    hsil = workp.tile([128, C], F32, tag="hsil")
                    nc.scalar.activation(
                        out=hsil[:], in_=h13[:, 0:C],
                        func=mybir.ActivationFunctionType.Sigmoid,
                    )
                    nc.vector.tensor_tensor(
                        out=hsil[:], in0=hsil[:], in1=h13[:, 0:C],
                        op=mybir.AluOpType.mult,
                    )
                    nc.vector.tensor_tensor(
                        out=hj[:, m, :], in0=hsil[:], in1=h13[:, C : 2 * C],
                        op=mybir.AluOpType.mult,
                    )

                # stage 2: out2 = h @ w2T  (out: [slot, H]) scaled by gating
                sc_t = workp.tile([128, C // 128, H], F32, tag="sc")
                for ci in range(C // 128):
                    for n2 in range(2):
                        o2 = psp.tile([128, 384], F32, tag="ps")
                        for k in range(ICH):
                            nc.tensor.matmul(
                                out=o2[:],
                                lhsT=hj[:, k, ci * 128 : (ci + 1) * 128],
                                rhs=w2[:, k, n2 * 384 : (n2 + 1) * 384],
                                start=(k == 0), stop=(k == ICH - 1),
                            )
                        nc.vector.tensor_scalar(
                            out=sc_t[:, ci, n2 * 384 : (n2 + 1) * 384],
                            in0=o2[:], scalar1=wg[:, ci, j : j + 1], scalar2=None,
                            op0=mybir.AluOpType.mult,
                        )

                nc.gpsimd.dma_scatter_add(
                    out_ap=acc_d[:, :], in_ap=sc_t[:],
                    idxs_ap=idxr[:, j * 16 : (j + 1) * 16],
                    num_idxs=C, num_idxs_reg=C, elem_size=H,
                )

            # ---------------- cross-core reduce ----------------
            nc.gpsimd.collective_compute(
                "ReduceScatter",
                mybir.AluOpType.add,
                replica_groups=[list(range(NCORES))],
                ins=[acc_d[0:T, :]],
                outs=[rs_d[:, :]],
            )
            nc.sync.dma_start(out=out_d[:, :], in_=rs_d[:, :])

    return nc


def make_core_inputs(inputs):
    """Host-side sharding: returns the per-core input maps (list of dicts)."""
    x = np.ascontiguousarray(np.asarray(inputs["hidden_states"], np.float32))
    gate_w = np.asarray(inputs["gate_w"], np.float32)
    e_bias = np.asarray(inputs["e_bias"], np.float32)
    w1 = np.asarray(inputs["w1"], np.float32)
    w3 = np.asarray(inputs["w3"], np.float32)
    w2 = np.asarray(inputs["w2"], np.float32)
    ws_gate = np.asarray(inputs["ws_gate"], np.float32)
    ws_up = np.asarray(inputs["ws_up"], np.float32)
    ws_down = np.asarray(inputs["ws_down"], np.float32)

    xT = np.ascontiguousarray(x.T)
    x_pad = np.zeros((T + 1, H), np.float32)
    x_pad[:T] = x
    tokid1 = (np.arange(T, dtype=np.float32) + 1.0).reshape(T, 1)
    slotidx = (
        np.arange(16, dtype=np.float32)[:, None]
        + 16.0 * np.arange(16, dtype=np.float32)[None, :]
    )  # slot(p, f) = f*16 + p
    maps = []
    for r in range(NCORES):
        rot = np.roll(np.arange(E), -EPC * r)
        mine = rot[:EPC]
        w13T = np.empty((EPC, H, 2 * I), np.float32)
        for jj, e in enumerate(mine):
            w13T[jj, :, :I] = w1[e].T
            w13T[jj, :, I:] = w3[e].T
        w2T = np.stack([np.ascontiguousarray(w2[e].T) for e in mine])
        sl = slice(r * SIPC, (r + 1) * SIPC)
        maps.append(
            {
                "xT": xT,
                "x_pad": x_pad,
                "gwT": np.ascontiguousarray(gate_w[rot].T),
                "ebias_b": np.broadcast_to(e_bias[rot], (128, E)).copy(),
                "w13T": w13T,
                "w2T": w2T,
                "wsgT": np.ascontiguousarray(ws_gate[sl].T),
                "wsuT": np.ascontiguousarray(ws_up[sl].T),
                "wsdT": np.ascontiguousarray(ws_down[:, sl].T),
                "tokid1": tokid1,
                "slotidx": slotidx,
            }
        )
    return maps


_NC_CACHE = None


def kernel(**inputs) -> np.ndarray:
    global _NC_CACHE
    if _NC_CACHE is None:
        nc = build_nc()
        nc.finalize()
        _NC_CACHE = nc
    nc = _NC_CACHE
    in_maps = make_core_inputs(inputs)
    res = run_bass_kernel_spmd(nc, in_maps, list(range(NCORES)))
    out = np.concatenate([res.results[i]["out"] for i in range(NCORES)], axis=0)
    return out.astype(np.float32)



# revision 9
# speedup vs baseline: 1.3222x; 1.3222x over previous
"""DeepseekV3-style MoE block on 8 Trainium2 NeuronCores (expert-parallel).

Sharding strategy (v2, bf16):
  - 64 routed experts sharded 8-per-core; expert columns rotated per core so
    each core's experts sit at columns 0..7 of its router output.
  - Router replicated, computed in f32r (selection must match reference).
  - Shared expert TP-sharded on intermediate dim (96 rows/core), bf16.
  - FFN weights/activations in bf16; PSUM accumulation f32.
  - Weights are host-packed so every SBUF partition line is one contiguous
    DRAM read (128 big DMA descriptors per tensor instead of ~768 row
    descriptors).
  - Token dispatch: dense combine weights -> per-expert compacted token lists
    AND compacted combine weights via two parallel gpsimd sparse_gathers ->
    dma_gather(transpose=True) pulls x rows straight into [H-part, slot]
    layout (no PE transposes) -> gated-MLP bf16 matmuls -> scale by combine
    weight -> dma_scatter_add(bf16) into dense accumulator.
  - ReduceScatter(add) in bf16 into a Shared-addr-space tensor; host concats
    the 8 shards and upcasts to f32.
"""

import numpy as np
import ml_dtypes

import concourse.bass as bass
import concourse.bacc as bacc
import concourse.mybir as mybir
import concourse.tile as tile
from concourse.bass_utils import run_bass_kernel_spmd
from concourse.masks import make_identity

F32 = mybir.dt.float32
F32R = mybir.dt.float32r
BF16 = mybir.dt.bfloat16
I16 = mybir.dt.int16
U32 = mybir.dt.uint32
U8 = mybir.dt.uint8

NPBF16 = ml_dtypes.bfloat16

# Model constants (hardcoded per contest rules)
E = 64          # experts
TOPK = 8
NG = 8          # groups
TOPKG = 4       # groups selected
SCALE = 2.5
H = 768         # hidden
I = 384         # routed expert intermediate
SI = 768        # shared expert intermediate
T = 1024        # tokens
NCORES = 8
EPC = E // NCORES     # experts per core = 8
SIPC = SI // NCORES   # shared-intermediate rows per core = 96
C = 256               # per-expert token capacity (max observed load is 224)
TCH = T // 128        # token chunks = 8
HCH = H // 128        # hidden chunks = 6
ICH = I // 128        # intermediate chunks = 3
BIG = 1.0e30


def r32(ap):
    return ap.bitcast(F32R)


def build_nc():
    nc = bacc.Bacc(num_devices=NCORES)

    # ---------------- I/O (all host-packed; see make_core_inputs) ----------
    # router inputs (f32)
    xTp_d = nc.declare_dram_parameter("xTp", [128, HCH * T], F32R, isOutput=False)
    gwp_d = nc.declare_dram_parameter("gwp", [128, HCH * E], F32R, isOutput=False)
    eb_d = nc.declare_dram_parameter("ebias_b", [128, E], F32, isOutput=False)
    tok_d = nc.declare_dram_parameter("tokid", [128, TCH], F32, isOutput=False)
    # bf16 x: row padded for gathers, packed-T for shared-expert rhs
    xbf_d = nc.declare_dram_parameter("x_bf", [T + 1, H], BF16, isOutput=False)
    xTbp_d = nc.declare_dram_parameter("xTbp", [128, HCH * T], BF16, isOutput=False)
    # routed expert weights, packed per partition line
    w13_d = nc.declare_dram_parameter("w13p", [EPC, 128, HCH * 2 * I], BF16, isOutput=False)
    w2_d = nc.declare_dram_parameter("w2p", [EPC, 128, ICH * H], BF16, isOutput=False)
    # shared expert (TP slice), packed
    wsg_d = nc.declare_dram_parameter("wsgp", [128, HCH * SIPC], BF16, isOutput=False)
    wsu_d = nc.declare_dram_parameter("wsup", [128, HCH * SIPC], BF16, isOutput=False)
    wsd_d = nc.declare_dram_parameter("wsdp", [SIPC, H], BF16, isOutput=False)
    out_d = nc.declare_dram_parameter("out", [T // NCORES, H], BF16, isOutput=True)

    # ---------------- internal DRAM ----------------
    vals_d = nc.dram_tensor("vals_d", [16, T], F32)        # rows 0..7 tokids, 8..15 weights
    idx_d = nc.dram_tensor("idx_d", [16, EPC * 16], I16)
    wv_d = nc.dram_tensor("wv_d", [128, 2 * EPC], F32)     # per-slot weights, scrambled
    acc_d = nc.dram_tensor("acc_d", [T + 1, H], BF16)
    rs_d = nc.dram_tensor("rs_d", [T // NCORES, H], BF16)

    with tile.TileContext(nc) as tc:
        with (
            tc.tile_pool(name="const", bufs=1) as constp,
            tc.tile_pool(name="xstream", bufs=2) as xsp,
            tc.tile_pool(name="wts", bufs=1) as wtsp,
            tc.tile_pool(name="route", bufs=1) as routep,
            tc.tile_pool(name="small", bufs=2) as smallp,
            tc.tile_pool(name="work", bufs=2) as workp,
            tc.tile_pool(name="psum", bufs=8, space="PSUM") as psp,
        ):
            # ---------------- constants / inputs ----------------
            ident = constp.tile([128, 128], F32, tag="ident")
            make_identity(nc, ident[:])
            ebias = constp.tile([128, E], F32, tag="ebias")
            nc.sync.dma_start(out=ebias[:], in_=eb_d[:, :])
            tokid = constp.tile([128, TCH], F32, tag="tokid")
            nc.sync.dma_start(out=tokid[:], in_=tok_d[:, :])
            ones1 = constp.tile([128, 1], F32, tag="ones1")
            nc.vector.memset(ones1[:], 1.0)

            gw = constp.tile([128, HCH, E], F32R, tag="gw")
            nc.sync.dma_start(out=gw[:], in_=gwp_d[:, :])
            xTb = constp.tile([128, HCH, T], BF16, tag="xTb")
            nc.sync.dma_start(out=xTb[:], in_=xTbp_d[:, :])
            wsg = constp.tile([128, HCH, SIPC], BF16, tag="wsg")
            nc.sync.dma_start(out=wsg[:], in_=wsg_d[:, :])
            wsu = constp.tile([128, HCH, SIPC], BF16, tag="wsu")
            nc.sync.dma_start(out=wsu[:], in_=wsu_d[:, :])
            wsd = constp.tile([SIPC, H], BF16, tag="wsd")
            nc.sync.dma_start(out=wsd[:], in_=wsd_d[:, :])

            # ---------------- router logits: logitsT = gw.T @ xT ------------
            # stream the f32 xT k-chunks; f32r matmuls (1 cyc/row at N=512)
            lgsb = routep.tile([64, T], F32, tag="lgsb")
            lgp0 = psp.tile([64, 512], F32, tag="ps")
            lgp1 = psp.tile([64, 512], F32, tag="ps")
            lgps = [lgp0, lgp1]
            for k in range(HCH):
                xck = xsp.tile([128, T], F32R, tag="xck")
                nc.sync.dma_start(out=xck[:], in_=xTp_d[:, k * T : (k + 1) * T])
                for n in range(2):
                    nc.tensor.matmul(
                        out=lgps[n][:],
                        lhsT=gw[:, k, :],
                        rhs=xck[:, n * 512 : (n + 1) * 512],
                        start=(k == 0),
                        stop=(k == HCH - 1),
                    )
            for n in range(2):
                nc.vector.tensor_copy(
                    out=lgsb[:, n * 512 : (n + 1) * 512], in_=lgps[n][:]
                )

            # ---------------- expert weight preload (all 8, bf16) -----------
            w13s, w2s = [], []
            for j in range(EPC):
                w13 = wtsp.tile([128, HCH, 2 * I], BF16, tag=f"w13_{j}")
                nc.sync.dma_start(out=w13[:], in_=w13_d[j])
                w2 = wtsp.tile([128, ICH, H], BF16, tag=f"w2_{j}")
                nc.sync.dma_start(out=w2[:], in_=w2_d[j])
                w13s.append(w13)
                w2s.append(w2)

            # ---------------- routing (batched DVE over all chunks) ---------
            scores = routep.tile([128, TCH, E], F32, tag="scores")
            for c in range(TCH):
                lt = psp.tile([128, 64], F32, tag="ps")
                nc.tensor.transpose(
                    out=lt[:], in_=lgsb[:, c * 128 : (c + 1) * 128],
                    identity=ident[:64, :64],
                )
                nc.scalar.activation(
                    out=scores[:, c, :], in_=lt[:],
                    func=mybir.ActivationFunctionType.Sigmoid,
                )

            swb = routep.tile([128, TCH, E], F32, tag="swb")
            nc.vector.tensor_tensor(
                out=swb[:], in0=scores[:],
                in1=ebias[:, None, :].to_broadcast([128, TCH, E]),
                op=mybir.AluOpType.add,
            )
            # group scores: top-2 sum per group of 8 (view [128, TCH*NG, 8])
            swg = swb[:].rearrange("p c (g e) -> p (c g) e", e=NG)
            m1 = routep.tile([128, TCH * NG], F32, tag="m1")
            nc.vector.tensor_reduce(
                out=m1[:], in_=swg, axis=mybir.AxisListType.X,
                op=mybir.AluOpType.max,
            )
            eq = routep.tile([128, TCH * NG, NG], F32, tag="eq")
            nc.vector.tensor_tensor(
                out=eq[:], in0=swg,
                in1=m1[:, :, None].to_broadcast([128, TCH * NG, NG]),
                op=mybir.AluOpType.is_equal,
            )
            nc.vector.tensor_scalar(
                out=eq[:], in0=eq[:], scalar1=-BIG, scalar2=None,
                op0=mybir.AluOpType.mult,
            )
            nc.vector.tensor_add(out=eq[:], in0=eq[:], in1=swg)
            m2 = routep.tile([128, TCH * NG], F32, tag="m2")
            nc.vector.tensor_reduce(
                out=m2[:], in_=eq[:], axis=mybir.AxisListType.X,
                op=mybir.AluOpType.max,
            )
            gs = routep.tile([128, TCH, NG], F32, tag="gs")
            nc.vector.tensor_add(
                out=gs[:].rearrange("p c g -> p (c g)"), in0=m1[:], in1=m2[:]
            )
            # top-4 groups per chunk
            g4s = routep.tile([128, TCH, 8], F32, tag="g4s")
            for c in range(TCH):
                nc.vector.max(out=g4s[:, c, :], in_=gs[:, c, :])
            gmask = routep.tile([128, TCH, NG], F32, tag="gmask")
            nc.vector.tensor_tensor(
                out=gmask[:], in0=gs[:],
                in1=g4s[:, :, TOPKG - 1 : TOPKG].to_broadcast([128, TCH, NG]),
                op=mybir.AluOpType.is_ge,
            )
            masked = routep.tile([128, TCH, E], F32, tag="masked")
            nc.vector.tensor_tensor(
                out=masked[:].rearrange("p c (g e) -> p (c g) e", e=NG),
                in0=swg,
                in1=gmask[:].rearrange("p c g -> p (c g)")[:, :, None]
                .to_broadcast([128, TCH * NG, NG]),
                op=mybir.AluOpType.mult,
            )
            # top-8 experts per chunk
            t8s = routep.tile([128, TCH, 8], F32, tag="t8s")
            for c in range(TCH):
                nc.vector.max(out=t8s[:, c, :], in_=masked[:, c, :])
            nmask = routep.tile([128, TCH, E], F32, tag="nmask")
            nc.vector.tensor_tensor(
                out=nmask[:], in0=masked[:],
                in1=t8s[:, :, TOPK - 1 : TOPK].to_broadcast([128, TCH, E]),
                op=mybir.AluOpType.is_ge,
            )
            sel = routep.tile([128, TCH, E], F32, tag="sel")
            nc.vector.tensor_tensor(
                out=sel[:], in0=scores[:], in1=nmask[:], op=mybir.AluOpType.mult
            )
            den = routep.tile([128, TCH], F32, tag="den")
            nc.vector.tensor_reduce(
                out=den[:], in_=sel[:], axis=mybir.AxisListType.X,
                op=mybir.AluOpType.add,
            )
            nc.vector.tensor_scalar(
                out=den[:], in0=den[:], scalar1=1e-20, scalar2=None,
                op0=mybir.AluOpType.add,
            )
            rec = routep.tile([128, TCH], F32, tag="rec")
            nc.vector.reciprocal(out=rec[:], in_=den[:])
            nc.vector.tensor_scalar(
                out=rec[:], in0=rec[:], scalar1=SCALE, scalar2=None,
                op0=mybir.AluOpType.mult,
            )
            nc.vector.tensor_tensor(
                out=sel[:], in0=sel[:],
                in1=rec[:, :, None].to_broadcast([128, TCH, E]),
                op=mybir.AluOpType.mult,
            )

            # my experts' (cols 0..7) selection mask, token-id vals and
            # weight vals for compaction
            mask8 = routep.tile([128, TCH, EPC], F32, tag="mask8")
            nc.vector.tensor_scalar(
                out=mask8[:], in0=sel[:, :, 0:EPC], scalar1=0.0, scalar2=None,
                op0=mybir.AluOpType.is_gt,
            )
            vw = routep.tile([128, TCH, 2 * EPC], F32, tag="vw")
            # vals: tokid where selected else -1
            nc.vector.tensor_tensor(
                out=vw[:, :, 0:EPC], in0=mask8[:],
                in1=tokid[:, :, None].to_broadcast([128, TCH, EPC]),
                op=mybir.AluOpType.mult,
            )
            nc.vector.tensor_scalar(
                out=vw[:, :, 0:EPC], in0=vw[:, :, 0:EPC], scalar1=-1.0,
                scalar2=None, op0=mybir.AluOpType.add,
            )
            # wvals: combine weight where selected else -1
            nc.vector.tensor_scalar(
                out=vw[:, :, EPC:], in0=sel[:, :, 0:EPC], scalar1=1.0,
                scalar2=None, op0=mybir.AluOpType.add,
            )
            nc.vector.tensor_tensor(
                out=vw[:, :, EPC:], in0=vw[:, :, EPC:], in1=mask8[:],
                op=mybir.AluOpType.mult,
            )
            nc.vector.tensor_scalar(
                out=vw[:, :, EPC:], in0=vw[:, :, EPC:], scalar1=-1.0,
                scalar2=None, op0=mybir.AluOpType.add,
            )

            # counts + transpose vals into [16, T] rows
            cnt_ps = psp.tile([EPC, 1], F32, tag="ps")
            valsT = routep.tile([16, T], F32, tag="valsT")
            for c in range(TCH):
                nc.tensor.matmul(
                    out=cnt_ps[:], lhsT=mask8[:, c, :], rhs=ones1[:],
                    start=(c == 0), stop=(c == TCH - 1),
                )
                vt = psp.tile([16, 128], F32, tag="ps")
                nc.tensor.transpose(out=vt[:], in_=vw[:, c, :], identity=ident[:])
                nc.vector.tensor_copy(
                    out=valsT[:, c * 128 : (c + 1) * 128], in_=vt[:]
                )

            # counts -> row on partition 0 -> broadcast to 16 partitions
            cnt = routep.tile([EPC, 1], F32, tag="cnt")
            nc.vector.tensor_copy(out=cnt[:], in_=cnt_ps[:])
            cntrow_ps = psp.tile([1, EPC], F32, tag="ps")
            nc.tensor.transpose(
                out=cntrow_ps[:], in_=cnt[:], identity=ident[:EPC, :EPC]
            )
            cntrow = routep.tile([1, EPC], F32, tag="cntrow")
            nc.vector.tensor_copy(out=cntrow[:], in_=cntrow_ps[:])
            cntb = routep.tile([16, EPC], F32, tag="cntb")
            nc.gpsimd.partition_broadcast(out_ap=cntb[:], in_ap=cntrow[:])

            # valsT -> DRAM -> 16-partition-wrapped view (single DMA back)
            nc.sync.dma_start(out=vals_d[:, :], in_=valsT[:])
            v16all = routep.tile([16, 2 * EPC, T // 16], F32, tag="v16all")
            nc.sync.dma_start(
                out=v16all[:],
                in_=bass.AP(vals_d, 0, [[1, 16], [T, 2 * EPC], [16, T // 16]]),
            )

            # compact per-expert token lists + weights (gpsimd sparse_gather)
            idxf = routep.tile([16, EPC * 16], F32, tag="idxf")
            wvf = routep.tile([16, EPC * 16], F32, tag="wvf")
            nfound = routep.tile([1, 2 * EPC], U32, tag="nfound")
            for j in range(EPC):
                nc.gpsimd.sparse_gather(
                    out=idxf[:, j * 16 : (j + 1) * 16],
                    in_=v16all[:, j, :],
                    num_found=nfound[:, j : j + 1],
                )
                nc.gpsimd.sparse_gather(
                    out=wvf[:, j * 16 : (j + 1) * 16],
                    in_=v16all[:, EPC + j, :],
                    num_found=nfound[:, EPC + j : EPC + j + 1],
                )

            # sanitize: slots >= count[j] -> token T (zero row / dummy row),
            # weight 0
            slotx = routep.tile([16, EPC, 16], I16, tag="slotx")
            nc.gpsimd.iota(
                slotx[:], pattern=[[0, EPC], [16, 16]], base=0,
                channel_multiplier=1,
            )
            slotxf = routep.tile([16, EPC, 16], F32, tag="slotxf")
            nc.vector.tensor_copy(out=slotxf[:], in_=slotx[:])
            keep = routep.tile([16, EPC, 16], U8, tag="keep")
            nc.vector.tensor_tensor(
                out=keep[:], in0=slotxf[:],
                in1=cntb[:, :, None].to_broadcast([16, EPC, 16]),
                op=mybir.AluOpType.is_lt,
            )
            padT = routep.tile([16, EPC, 16], F32, tag="padT")
            nc.vector.memset(padT[:], float(T))
            zero16 = routep.tile([16, EPC, 16], F32, tag="zero16")
            nc.vector.memset(zero16[:], 0.0)
            idxs = routep.tile([16, EPC, 16], F32, tag="idxs")
            nc.vector.select(
                out=idxs[:], mask=keep[:],
                on_true=idxf[:].rearrange("p (j f) -> p j f", f=16),
                on_false=padT[:],
            )
            wvs = routep.tile([16, EPC, 16], F32, tag="wvs")
            nc.vector.select(
                out=wvs[:], mask=keep[:],
                on_true=wvf[:].rearrange("p (j f) -> p j f", f=16),
                on_false=zero16[:],
            )
            idx16 = routep.tile([16, EPC * 16], I16, tag="idx16")
            nc.vector.tensor_copy(
                out=idx16[:].rearrange("p (j f) -> p j f", f=16), in_=idxs[:]
            )
            # replicate idx rows to all 8 16-partition groups (DRAM bounce)
            nc.sync.dma_start(out=idx_d[:, :], in_=idx16[:])
            idxr = routep.tile([128, EPC * 16], I16, tag="idxr")
            nc.sync.dma_start(
                out=idxr[:],
                in_=bass.AP(idx_d, 0, [[0, 8], [EPC * 16, 16], [1, EPC * 16]]),
            )
            # weights -> DRAM with scatter AP so the read back is
            # wcol[p, j*2+ci] = w_j[slot ci*128+p]
            nc.sync.dma_start(
                out=bass.AP(wv_d, 0, [[16, 16], [2, EPC], [1, 2], [256, 8]]),
                in_=wvs[:],
            )
            wcol = routep.tile([128, 2 * EPC], F32, tag="wcol")
            nc.sync.dma_start(out=wcol[:], in_=wv_d[:, :])

            # ---------------- shared expert (TP slice, bf16) ----------------
            hsh = routep.tile([SIPC, T], BF16, tag="hsh")
            for n in range(2):
                hg = psp.tile([SIPC, 512], F32, tag="ps")
                hu = psp.tile([SIPC, 512], F32, tag="ps")
                for k in range(HCH):
                    nc.tensor.matmul(
                        out=hg[:], lhsT=wsg[:, k, :],
                        rhs=xTb[:, k, n * 512 : (n + 1) * 512],
                        start=(k == 0), stop=(k == HCH - 1),
                    )
                for k in range(HCH):
                    nc.tensor.matmul(
                        out=hu[:], lhsT=wsu[:, k, :],
                        rhs=xTb[:, k, n * 512 : (n + 1) * 512],
                        start=(k == 0), stop=(k == HCH - 1),
                    )
                hsig = smallp.tile([SIPC, 512], F32, tag="hsig")
                nc.scalar.activation(
                    out=hsig[:], in_=hg[:],
                    func=mybir.ActivationFunctionType.Sigmoid,
                )
                nc.vector.tensor_tensor(
                    out=hsig[:], in0=hsig[:], in1=hg[:], op=mybir.AluOpType.mult
                )
                nc.vector.tensor_tensor(
                    out=hsh[:, n * 512 : (n + 1) * 512], in0=hsig[:], in1=hu[:],
                    op=mybir.AluOpType.mult,
                )
            for c in range(TCH):
                so = workp.tile([128, H], BF16, tag="so")
                for n2 in range(2):
                    sp = psp.tile([128, 384], F32, tag="ps")
                    nc.tensor.matmul(
                        out=sp[:],
                        lhsT=hsh[:, c * 128 : (c + 1) * 128],
                        rhs=wsd[:, n2 * 384 : (n2 + 1) * 384],
                        start=True, stop=True,
                    )
                    nc.vector.tensor_copy(
                        out=so[:, n2 * 384 : (n2 + 1) * 384], in_=sp[:]
                    )
                nc.sync.dma_start(out=acc_d[c * 128 : (c + 1) * 128, :], in_=so[:])

            # ---------------- routed experts (bf16) ----------------
            for j in range(EPC):
                # gather x rows straight into transposed [H-part, slot] layout
                xgT = workp.tile([128, HCH, C], BF16, tag="xgT")
                nc.gpsimd.dma_gather(
                    out_ap=xgT[:], in_ap=xbf_d[:, :],
                    idxs_ap=idxr[:, j * 16 : (j + 1) * 16],
                    num_idxs=C, num_idxs_reg=C, elem_size=H,
                    transpose=True,
                )

                # stage 1: h = silu(w1 @ xgT) * (w3 @ xgT)   [I, C]
                w13 = w13s[j]
                hj = workp.tile([128, ICH, C], BF16, tag="hj")
                for m in range(ICH):
                    h13 = psp.tile([128, 512], F32, tag="ps")
                    for k in range(HCH):
                        nc.tensor.matmul(
                            out=h13[:, 0:C],
                            lhsT=w13[:, k, m * 128 : (m + 1) * 128],
                            rhs=xgT[:, k, :],
                            start=(k == 0), stop=(k == HCH - 1),
                        )
                    for k in range(HCH):
                        nc.tensor.matmul(
                            out=h13[:, C : 2 * C],
                            lhsT=w13[:, k, I + m * 128 : I + (m + 1) * 128],
                            rhs=xgT[:, k, :],
                            start=(k == 0), stop=(k == HCH - 1),
                        )
                    hsil = workp.tile([128, C], F32, tag="hsil")
                    nc.scalar.activation(
                        out=hsil[:], in_=h13[:, 0:C],
                        func=mybir.ActivationFunctionType.Sigmoid,
                    )
                    nc.vector.tensor_tensor(
                        out=hsil[:], in0=hsil[:], in1=h13[:, 0:C],
                        op=mybir.AluOpType.mult,
                    )
                    nc.vector.tensor_tensor(
                        out=hj[:, m, :], in0=hsil[:], in1=h13[:, C : 2 * C],
                        op=mybir.AluOpType.mult,
                    )

                # stage 2: y = (h @ w2T) * w_slot  ->  [slot, H]
                w2 = w2s[j]
                sc_t = workp.tile([128, C // 128, H], BF16, tag="sc")
                for ci in range(C // 128):
                    for n2 in range(2):
                        o2 = psp.tile([128, 384], F32, tag="ps")
                        for k in range(ICH):
                            nc.tensor.matmul(
                                out=o2[:],
                                lhsT=hj[:, k, ci * 128 : (ci + 1) * 128],
                                rhs=w2[:, k, n2 * 384 : (n2 + 1) * 384],
                                start=(k == 0), stop=(k == ICH - 1),
                            )
                        nc.vector.tensor_scalar(
                            out=sc_t[:, ci, n2 * 384 : (n2 + 1) * 384],
                            in0=o2[:],
                            scalar1=wcol[:, j * 2 + ci : j * 2 + ci + 1],
                            scalar2=None,
                            op0=mybir.AluOpType.mult,
                        )

                nc.gpsimd.dma_scatter_add(
                    out_ap=acc_d[:, :], in_ap=sc_t[:],
                    idxs_ap=idxr[:, j * 16 : (j + 1) * 16],
                    num_idxs=C, num_idxs_reg=C, elem_size=H,
                )

            # ---------------- cross-core reduce (bf16) ----------------
            nc.gpsimd.collective_compute(
                "ReduceScatter",
                mybir.AluOpType.add,
                replica_groups=[list(range(NCORES))],
                ins=[acc_d[0:T, :]],
                outs=[rs_d[:, :]],
            )
            nc.sync.dma_start(out=out_d[:, :], in_=rs_d[:, :])

    return nc


def _pack_kT(a, dtype):
    """[H, N] -> [128, HCH*N] so each partition line is contiguous in DRAM.

    Element (p, k*N + t) = a[k*128 + p, t].
    """
    Hh, N = a.shape
    kch = Hh // 128
    return np.ascontiguousarray(
        a.reshape(kch, 128, N).transpose(1, 0, 2).reshape(128, kch * N)
    ).astype(dtype)


def make_core_inputs(inputs):
    """Host-side sharding: returns the per-core input maps (list of dicts)."""
    x = np.asarray(inputs["hidden_states"], np.float32)
    gate_w = np.asarray(inputs["gate_w"], np.float32)
    e_bias = np.asarray(inputs["e_bias"], np.float32)
    w1 = np.asarray(inputs["w1"], np.float32)
    w3 = np.asarray(inputs["w3"], np.float32)
    w2 = np.asarray(inputs["w2"], np.float32)
    ws_gate = np.asarray(inputs["ws_gate"], np.float32)
    ws_up = np.asarray(inputs["ws_up"], np.float32)
    ws_down = np.asarray(inputs["ws_down"], np.float32)

    xT = np.ascontiguousarray(x.T)                      # [H, T]
    xTp = _pack_kT(xT, np.float32)
    xTbp = _pack_kT(xT, NPBF16)
    x_bf = np.zeros((T + 1, H), NPBF16)
    x_bf[:T] = x.astype(NPBF16)
    tokid = (
        np.arange(128, dtype=np.float32)[:, None]
        + 128.0 * np.arange(TCH, dtype=np.float32)[None, :]
        + 1.0
    )  # (p, c) -> c*128 + p + 1

    maps = []
    for r in range(NCORES):
        rot = np.roll(np.arange(E), -EPC * r)
        mine = rot[:EPC]
        w13p = np.empty((EPC, 128, HCH * 2 * I), NPBF16)
        w2p = np.empty((EPC, 128, ICH * H), NPBF16)
        for jj, e in enumerate(mine):
            w13T = np.concatenate([w1[e].T, w3[e].T], axis=1)  # [H, 2I]
            w13p[jj] = _pack_kT(w13T, NPBF16)
            w2p[jj] = _pack_kT(np.ascontiguousarray(w2[e].T), NPBF16)
        sl = slice(r * SIPC, (r + 1) * SIPC)
        maps.append(
            {
                "xTp": xTp,
                "xTbp": xTbp,
                "x_bf": x_bf,
                "gwp": _pack_kT(np.ascontiguousarray(gate_w[rot].T), np.float32),
                "ebias_b": np.broadcast_to(e_bias[rot], (128, E)).copy(),
                "w13p": w13p,
                "w2p": w2p,
                "wsgp": _pack_kT(np.ascontiguousarray(ws_gate[sl].T), NPBF16),
                "wsup": _pack_kT(np.ascontiguousarray(ws_up[sl].T), NPBF16),
                "wsdp": np.ascontiguousarray(ws_down[:, sl].T).astype(NPBF16),
                "tokid": tokid,
            }
        )
    return maps


_NC_CACHE = None


def kernel(**inputs) -> np.ndarray:
    global _NC_CACHE
    if _NC_CACHE is None:
        nc = build_nc()
        nc.finalize()
        _NC_CACHE = nc
    nc = _NC_CACHE
    in_maps = make_core_inputs(inputs)
    res = run_bass_kernel_spmd(nc, in_maps, list(range(NCORES)))
    out = np.concatenate([res.results[i]["out"] for i in range(NCORES)], axis=0)
    return out.astype(np.float32)


# revision 31
# speedup vs baseline: 1.4249x; 1.0777x over previous
"""DeepseekV3-style MoE block on 8 Trainium2 NeuronCores (expert-parallel).

Sharding strategy (v3, bf16 + matmul combine):
  - 64 routed experts sharded 8-per-core; expert columns rotated per core so
    each core's experts sit at columns 0..7 of its router output.
  - Router replicated, computed in f32r (selection must match reference).
  - Shared expert TP-sharded on intermediate dim (96 rows/core), bf16.
  - FFN weights/activations bf16; PSUM accumulation f32. Weights host-packed
    so every SBUF partition line is one contiguous DRAM read.
  - Token dispatch: dense combine weights -> per-expert compacted token lists
    AND compacted combine weights via parallel gpsimd sparse_gathers ->
    dma_gather(transpose=True) pulls x rows straight into [H-part, slot]
    layout (pad slots read the zero row T).
  - Token combine: NO scatter-add. Per expert a one-hot matrix
    Pw_j[slot, tok] = w_j[slot] * (tok == token(slot)) is built on the DVE;
    out[tok, :] = shared + sum_j Pw_j^T @ y_j accumulates in PSUM, 17
    matmuls per output chunk. Pad slots have token id T so they never match.
  - ReduceScatter(add) in bf16; host concats the 8 shards and upcasts.
"""

import numpy as np
import ml_dtypes

import concourse.bass as bass
import concourse.bacc as bacc
import concourse.mybir as mybir
import concourse.tile as tile
from concourse.bass_utils import run_bass_kernel_spmd
from concourse.masks import make_identity

F32 = mybir.dt.float32
F32R = mybir.dt.float32r
BF16 = mybir.dt.bfloat16
I16 = mybir.dt.int16
U32 = mybir.dt.uint32
U8 = mybir.dt.uint8

NPBF16 = ml_dtypes.bfloat16

# Model constants (hardcoded per contest rules)
E = 64          # experts
TOPK = 8
NG = 8          # groups
TOPKG = 4       # groups selected
SCALE = 2.5
H = 768         # hidden
I = 384         # routed expert intermediate
SI = 768        # shared expert intermediate
T = 1024        # tokens
NCORES = 8
EPC = E // NCORES     # experts per core = 8
SIPC = SI // NCORES   # shared-intermediate rows per core = 96
C = 256               # per-expert token capacity (max observed load is 224)
TCH = T // 128        # token chunks = 8
HCH = H // 128        # hidden chunks = 6
ICH = I // 128        # intermediate chunks = 3
BIG = 1.0e30


def build_nc():
    nc = bacc.Bacc(num_devices=NCORES)

    # ---------------- I/O (all host-packed; see make_core_inputs) ----------
    xTp_d = nc.declare_dram_parameter("xTp", [128, HCH * T], F32R, isOutput=False)
    gwp_d = nc.declare_dram_parameter("gwp", [128, HCH * E], F32R, isOutput=False)
    eb_d = nc.declare_dram_parameter("ebias_b", [128, E], F32, isOutput=False)
    tok_d = nc.declare_dram_parameter("tokid", [128, TCH], F32, isOutput=False)
    slot_d = nc.declare_dram_parameter("slotc", [16, 16], F32, isOutput=False)
    iota_d = nc.declare_dram_parameter("iotab", [128, T], F32, isOutput=False)
    xbf_d = nc.declare_dram_parameter("x_bf", [T + 1, H], BF16, isOutput=False)
    xTbp_d = nc.declare_dram_parameter("xTbp", [128, HCH * T], BF16, isOutput=False)
    w13_d = nc.declare_dram_parameter("w13p", [EPC, 128, HCH * 2 * I], BF16, isOutput=False)
    w2_d = nc.declare_dram_parameter("w2p", [EPC, 128, ICH * H], BF16, isOutput=False)
    wsg_d = nc.declare_dram_parameter("wsgp", [128, HCH * SIPC], BF16, isOutput=False)
    wsu_d = nc.declare_dram_parameter("wsup", [128, HCH * SIPC], BF16, isOutput=False)
    wsd_d = nc.declare_dram_parameter("wsdp", [SIPC, H], BF16, isOutput=False)
    out_d = nc.declare_dram_parameter("out", [T // NCORES, H], BF16, isOutput=True)

    # ---------------- internal DRAM ----------------
    vals_d = nc.dram_tensor("vals_d", [16, T], F32)    # rows 0..7 tokids, 8..15 weights
    wv_d = nc.dram_tensor("wv_d", [128, 2 * EPC], F32)   # per-slot weights, scrambled
    iv_d = nc.dram_tensor("iv_d", [128, 2 * EPC], F32)   # per-slot token ids, scrambled
    idx_d = nc.dram_tensor("idx_d", [16, EPC * 16], I16)
    acc_d = nc.dram_tensor("acc_d", [T, H], BF16)
    rs_d = nc.dram_tensor("rs_d", [T // NCORES, H], BF16)

    with tile.TileContext(nc) as tc:
        with (
            tc.tile_pool(name="const", bufs=1) as constp,
            tc.tile_pool(name="xstream", bufs=2) as xsp,
            tc.tile_pool(name="wts", bufs=2) as wtsp,
            tc.tile_pool(name="route", bufs=1) as routep,
            tc.tile_pool(name="keep", bufs=1) as keepp,
            tc.tile_pool(name="small", bufs=2) as smallp,
            tc.tile_pool(name="work", bufs=2) as workp,
            tc.tile_pool(name="psum", bufs=8, space="PSUM") as psp,
        ):
            # ---------------- constants / inputs ----------------
            ident = constp.tile([128, 128], F32, tag="ident")
            make_identity(nc, ident[:])
            ebias = constp.tile([128, E], F32, tag="ebias")
            nc.sync.dma_start(out=ebias[:], in_=eb_d[:, :])
            tokid = constp.tile([128, TCH], F32, tag="tokid")
            nc.sync.dma_start(out=tokid[:], in_=tok_d[:, :])
            slotc = constp.tile([16, 16], F32, tag="slotc")
            nc.sync.dma_start(out=slotc[:], in_=slot_d[:, :])
            iotab = constp.tile([128, T], F32, tag="iotab")
            nc.sync.dma_start(out=iotab[:], in_=iota_d[:, :])

            gw = constp.tile([128, HCH, E], F32R, tag="gw")
            nc.sync.dma_start(out=gw[:], in_=gwp_d[:, :])
            xTb = constp.tile([128, HCH, T], BF16, tag="xTb")
            nc.sync.dma_start(out=xTb[:], in_=xTbp_d[:, :])
            wsg = constp.tile([128, HCH, SIPC], BF16, tag="wsg")
            nc.sync.dma_start(out=wsg[:], in_=wsg_d[:, :])
            wsu = constp.tile([128, HCH, SIPC], BF16, tag="wsu")
            nc.sync.dma_start(out=wsu[:], in_=wsu_d[:, :])
            wsd = constp.tile([SIPC, H], BF16, tag="wsd")
            nc.sync.dma_start(out=wsd[:], in_=wsd_d[:, :])

            # ---------------- router logits: logitsT = gw.T @ xT ------------
            lgsb = routep.tile([64, T], F32, tag="lgsb")
            lgp0 = psp.tile([64, 512], F32, tag="ps")
            lgp1 = psp.tile([64, 512], F32, tag="ps")
            lgps = [lgp0, lgp1]
            for k in range(HCH):
                xck = xsp.tile([128, T], F32R, tag="xck")
                nc.sync.dma_start(out=xck[:], in_=xTp_d[:, k * T : (k + 1) * T])
                for n in range(2):
                    nc.tensor.matmul(
                        out=lgps[n][:],
                        lhsT=gw[:, k, :],
                        rhs=xck[:, n * 512 : (n + 1) * 512],
                        start=(k == 0),
                        stop=(k == HCH - 1),
                    )
            for n in range(2):
                nc.vector.tensor_copy(
                    out=lgsb[:, n * 512 : (n + 1) * 512], in_=lgps[n][:]
                )

            # ---------------- routing (batched DVE over all chunks) ---------
            scores = routep.tile([128, TCH, E], F32, tag="scores")
            for c in range(TCH):
                lt = psp.tile([128, 64], F32, tag="ps")
                nc.tensor.transpose(
                    out=lt[:], in_=lgsb[:, c * 128 : (c + 1) * 128],
                    identity=ident[:64, :64],
                )
                nc.scalar.activation(
                    out=scores[:, c, :], in_=lt[:],
                    func=mybir.ActivationFunctionType.Sigmoid,
                )

            swb = routep.tile([128, TCH, E], F32, tag="swb")
            nc.vector.tensor_tensor(
                out=swb[:], in0=scores[:],
                in1=ebias[:, None, :].to_broadcast([128, TCH, E]),
                op=mybir.AluOpType.add,
            )
            swg = swb[:].rearrange("p c (g e) -> p (c g) e", e=NG)
            m1 = routep.tile([128, TCH * NG], F32, tag="m1")
            nc.vector.tensor_reduce(
                out=m1[:], in_=swg, axis=mybir.AxisListType.X,
                op=mybir.AluOpType.max,
            )
            eq = routep.tile([128, TCH * NG, NG], F32, tag="eq")
            nc.vector.tensor_tensor(
                out=eq[:], in0=swg,
                in1=m1[:, :, None].to_broadcast([128, TCH * NG, NG]),
                op=mybir.AluOpType.is_equal,
            )
            nc.vector.tensor_scalar(
                out=eq[:], in0=eq[:], scalar1=-BIG, scalar2=None,
                op0=mybir.AluOpType.mult,
            )
            nc.vector.tensor_add(out=eq[:], in0=eq[:], in1=swg)
            m2 = routep.tile([128, TCH * NG], F32, tag="m2")
            nc.vector.tensor_reduce(
                out=m2[:], in_=eq[:], axis=mybir.AxisListType.X,
                op=mybir.AluOpType.max,
            )
            gs = routep.tile([128, TCH, NG], F32, tag="gs")
            nc.vector.tensor_add(
                out=gs[:].rearrange("p c g -> p (c g)"), in0=m1[:], in1=m2[:]
            )
            g4s = routep.tile([128, TCH, 8], F32, tag="g4s")
            for c in range(TCH):
                nc.vector.max(out=g4s[:, c, :], in_=gs[:, c, :])
            gmask = routep.tile([128, TCH, NG], F32, tag="gmask")
            nc.vector.tensor_tensor(
                out=gmask[:], in0=gs[:],
                in1=g4s[:, :, TOPKG - 1 : TOPKG].to_broadcast([128, TCH, NG]),
                op=mybir.AluOpType.is_ge,
            )
            masked = routep.tile([128, TCH, E], F32, tag="masked")
            nc.vector.tensor_tensor(
                out=masked[:].rearrange("p c (g e) -> p (c g) e", e=NG),
                in0=swg,
                in1=gmask[:].rearrange("p c g -> p (c g)")[:, :, None]
                .to_broadcast([128, TCH * NG, NG]),
                op=mybir.AluOpType.mult,
            )
            t8s = routep.tile([128, TCH, 8], F32, tag="t8s")
            for c in range(TCH):
                nc.vector.max(out=t8s[:, c, :], in_=masked[:, c, :])
            nmask = routep.tile([128, TCH, E], F32, tag="nmask")
            nc.vector.tensor_tensor(
                out=nmask[:], in0=masked[:],
                in1=t8s[:, :, TOPK - 1 : TOPK].to_broadcast([128, TCH, E]),
                op=mybir.AluOpType.is_ge,
            )
            sel = routep.tile([128, TCH, E], F32, tag="sel")
            nc.vector.tensor_tensor(
                out=sel[:], in0=scores[:], in1=nmask[:], op=mybir.AluOpType.mult
            )
            den = routep.tile([128, TCH], F32, tag="den")
            nc.vector.tensor_reduce(
                out=den[:], in_=sel[:], axis=mybir.AxisListType.X,
                op=mybir.AluOpType.add,
            )
            nc.vector.tensor_scalar(
                out=den[:], in0=den[:], scalar1=1e-20, scalar2=None,
                op0=mybir.AluOpType.add,
            )
            rec = routep.tile([128, TCH], F32, tag="rec")
            nc.vector.reciprocal(out=rec[:], in_=den[:])
            nc.vector.tensor_scalar(
                out=rec[:], in0=rec[:], scalar1=SCALE, scalar2=None,
                op0=mybir.AluOpType.mult,
            )
            nc.vector.tensor_tensor(
                out=sel[:], in0=sel[:],
                in1=rec[:, :, None].to_broadcast([128, TCH, E]),
                op=mybir.AluOpType.mult,
            )

            # my experts' (cols 0..7) selection mask + compaction values
            mask8 = routep.tile([128, TCH, EPC], F32, tag="mask8")
            nc.vector.tensor_scalar(
                out=mask8[:], in0=sel[:, :, 0:EPC], scalar1=0.0, scalar2=None,
                op0=mybir.AluOpType.is_gt,
            )
            vw = routep.tile([128, TCH, 2 * EPC], F32, tag="vw")
            nc.vector.tensor_tensor(
                out=vw[:, :, 0:EPC], in0=mask8[:],
                in1=tokid[:, :, None].to_broadcast([128, TCH, EPC]),
                op=mybir.AluOpType.mult,
            )
            nc.vector.tensor_scalar(
                out=vw[:, :, 0:EPC], in0=vw[:, :, 0:EPC], scalar1=-1.0,
                scalar2=None, op0=mybir.AluOpType.add,
            )
            nc.vector.tensor_scalar(
                out=vw[:, :, EPC:], in0=sel[:, :, 0:EPC], scalar1=1.0,
                scalar2=None, op0=mybir.AluOpType.add,
            )
            nc.vector.tensor_tensor(
                out=vw[:, :, EPC:], in0=vw[:, :, EPC:], in1=mask8[:],
                op=mybir.AluOpType.mult,
            )
            nc.vector.tensor_scalar(
                out=vw[:, :, EPC:], in0=vw[:, :, EPC:], scalar1=-1.0,
                scalar2=None, op0=mybir.AluOpType.add,
            )

            valsT = routep.tile([16, T], F32, tag="valsT")
            for c in range(TCH):
                vt = psp.tile([16, 128], F32, tag="ps")
                nc.tensor.transpose(out=vt[:], in_=vw[:, c, :], identity=ident[:])
                nc.vector.tensor_copy(
                    out=valsT[:, c * 128 : (c + 1) * 128], in_=vt[:]
                )

            # valsT -> DRAM -> 16-partition-wrapped view (wrap t = p*64 + f
            # keeps partition lines contiguous; wrap order is irrelevant to
            # the compaction)
            nc.sync.dma_start(out=vals_d[:, :], in_=valsT[:])
            v16all = routep.tile([16, 2 * EPC, T // 16], F32, tag="v16all")
            nc.sync.dma_start(
                out=v16all[:],
                in_=bass.AP(vals_d, 0, [[T // 16, 16], [T, 2 * EPC], [1, T // 16]]),
            )

            # compact per-expert token lists + weights (gpsimd sparse_gather)
            idxf = routep.tile([16, EPC * 16], F32, tag="idxf")
            wvs = routep.tile([16, EPC, 16], F32, tag="wvs")
            nfound = routep.tile([1, 2 * EPC], U32, tag="nfound")
            nc.vector.memset(idxf[:], -1.0)
            nc.vector.memset(wvs[:], 0.0)
            for j in range(EPC):
                nc.gpsimd.sparse_gather(
                    out=idxf[:, j * 16 : (j + 1) * 16],
                    in_=v16all[:, j, :],
                    num_found=nfound[:, j : j + 1],
                )
                nc.gpsimd.sparse_gather(
                    out=wvs[:, j, :],
                    in_=v16all[:, EPC + j, :],
                    num_found=nfound[:, EPC + j : EPC + j + 1],
                )
            # sanitize: sparse_gather writes ARBITRARY (possibly NaN) values
            # beyond num_found on hardware, so pads must be replaced via
            # select() (NaN-garbage-proof) using slot < count masks.
            cntf = routep.tile([1, 2 * EPC], F32, tag="cntf")
            nc.vector.tensor_copy(out=cntf[:], in_=nfound[:])
            cntb = routep.tile([16, 2 * EPC], F32, tag="cntb")
            nc.gpsimd.partition_broadcast(out_ap=cntb[:], in_ap=cntf[:])
            padT = routep.tile([16, 16], F32, tag="padT")
            nc.vector.memset(padT[:], float(T))
            zero16 = routep.tile([16, 16], F32, tag="zero16")
            nc.vector.memset(zero16[:], 0.0)
            idxt = routep.tile([16, EPC * 16], F32, tag="idxt")
            wvc = routep.tile([16, EPC, 16], F32, tag="wvc")
            for j in range(EPC):
                keep = smallp.tile([16, 16], U8, tag="keep")
                nc.vector.tensor_scalar(
                    out=keep[:], in0=slotc[:], scalar1=cntb[:, j : j + 1],
                    scalar2=None, op0=mybir.AluOpType.is_lt,
                )
                nc.vector.select(
                    out=idxt[:, j * 16 : (j + 1) * 16], mask=keep[:],
                    on_true=idxf[:, j * 16 : (j + 1) * 16], on_false=padT[:],
                )
                nc.vector.select(
                    out=wvc[:, j, :], mask=keep[:],
                    on_true=wvs[:, j, :], on_false=zero16[:],
                )
            idxt16 = routep.tile([16, EPC * 16], I16, tag="idxt16")
            nc.vector.tensor_copy(out=idxt16[:], in_=idxt[:])

            # replicate idx rows to all 8 16-partition groups (DRAM bounce)
            nc.sync.dma_start(out=idx_d[:, :], in_=idxt16[:])
            idxr = routep.tile([128, EPC * 16], I16, tag="idxr")
            nc.sync.dma_start(
                out=idxr[:],
                in_=bass.AP(idx_d, 0, [[0, 8], [EPC * 16, 16], [1, EPC * 16]]),
            )
            # weights and token ids -> DRAM with scatter AP so the read back
            # is col[p, j*2+ci] = value at slot ci*128+p of expert j
            scr_ap = [[16, 16], [2, EPC], [1, 2], [256, 8]]
            nc.sync.dma_start(out=bass.AP(wv_d, 0, scr_ap), in_=wvc[:])
            nc.sync.dma_start(
                out=bass.AP(iv_d, 0, scr_ap),
                in_=idxt[:].rearrange("p (j f) -> p j f", f=16),
            )
            wcol = routep.tile([128, 2 * EPC], F32, tag="wcol")
            nc.sync.dma_start(out=wcol[:], in_=wv_d[:, :])
            icol = routep.tile([128, 2 * EPC], F32, tag="icol")
            nc.sync.dma_start(out=icol[:], in_=iv_d[:, :])

            # one-hot combine matrices: Pw_j[p, ci, t] =
            #   w_j[slot ci*128+p] * (t == token(slot ci*128+p))
            pws = []
            for j in range(EPC):
                pw = keepp.tile([128, 2, T], BF16, tag=f"pw_{j}")
                for ci in range(2):
                    nc.vector.tensor_scalar(
                        out=pw[:, ci, :], in0=iotab[:],
                        scalar1=icol[:, j * 2 + ci : j * 2 + ci + 1],
                        scalar2=None, op0=mybir.AluOpType.is_equal,
                    )
                    nc.vector.tensor_scalar(
                        out=pw[:, ci, :], in0=pw[:, ci, :],
                        scalar1=wcol[:, j * 2 + ci : j * 2 + ci + 1],
                        scalar2=None, op0=mybir.AluOpType.mult,
                    )
                pws.append(pw)

            # ---------------- shared expert stage 1 (TP slice, bf16) --------
            hsh = routep.tile([SIPC, T], BF16, tag="hsh")
            for n in range(2):
                hg = psp.tile([SIPC, 512], F32, tag="ps")
                hu = psp.tile([SIPC, 512], F32, tag="ps")
                for k in range(HCH):
                    nc.tensor.matmul(
                        out=hg[:], lhsT=wsg[:, k, :],
                        rhs=xTb[:, k, n * 512 : (n + 1) * 512],
                        start=(k == 0), stop=(k == HCH - 1),
                    )
                for k in range(HCH):
                    nc.tensor.matmul(
                        out=hu[:], lhsT=wsu[:, k, :],
                        rhs=xTb[:, k, n * 512 : (n + 1) * 512],
                        start=(k == 0), stop=(k == HCH - 1),
                    )
                hsig = smallp.tile([SIPC, 512], F32, tag="hsig")
                nc.scalar.activation(
                    out=hsig[:], in_=hg[:],
                    func=mybir.ActivationFunctionType.Sigmoid,
                )
                nc.vector.tensor_tensor(
                    out=hsig[:], in0=hsig[:], in1=hg[:], op=mybir.AluOpType.mult
                )
                nc.vector.tensor_tensor(
                    out=hsh[:, n * 512 : (n + 1) * 512], in0=hsig[:], in1=hu[:],
                    op=mybir.AluOpType.mult,
                )

            # ---------------- routed experts (bf16) ----------------
            ys = []
            for j in range(EPC):
                w13 = wtsp.tile([128, HCH, 2 * I], BF16, tag="w13")
                nc.sync.dma_start(out=w13[:], in_=w13_d[j])
                w2 = wtsp.tile([128, ICH, H], BF16, tag="w2")
                nc.sync.dma_start(out=w2[:], in_=w2_d[j])

                xgT = workp.tile([128, HCH, C], BF16, tag="xgT")
                nc.gpsimd.dma_gather(
                    out_ap=xgT[:], in_ap=xbf_d[:, :],
                    idxs_ap=idxr[:, j * 16 : (j + 1) * 16],
                    num_idxs=C, num_idxs_reg=C, elem_size=H,
                    transpose=True,
                )

                hj = workp.tile([128, ICH, C], BF16, tag="hj")
                for m in range(ICH):
                    h13 = psp.tile([128, 512], F32, tag="ps")
                    for k in range(HCH):
                        nc.tensor.matmul(
                            out=h13[:, 0:C],
                            lhsT=w13[:, k, m * 128 : (m + 1) * 128],
                            rhs=xgT[:, k, :],
                            start=(k == 0), stop=(k == HCH - 1),
                        )
                    for k in range(HCH):
                        nc.tensor.matmul(
                            out=h13[:, C : 2 * C],
                            lhsT=w13[:, k, I + m * 128 : I + (m + 1) * 128],
                            rhs=xgT[:, k, :],
                            start=(k == 0), stop=(k == HCH - 1),
                        )
                    hsil = workp.tile([128, C], F32, tag="hsil")
                    nc.scalar.activation(
                        out=hsil[:], in_=h13[:, 0:C],
                        func=mybir.ActivationFunctionType.Sigmoid,
                    )
                    nc.vector.tensor_tensor(
                        out=hsil[:], in0=hsil[:], in1=h13[:, 0:C],
                        op=mybir.AluOpType.mult,
                    )
                    nc.vector.tensor_tensor(
                        out=hj[:, m, :], in0=hsil[:], in1=h13[:, C : 2 * C],
                        op=mybir.AluOpType.mult,
                    )

                y = keepp.tile([128, C // 128, H], BF16, tag=f"y_{j}")
                for ci in range(C // 128):
                    for n2 in range(2):
                        o2 = psp.tile([128, 384], F32, tag="ps")
                        for k in range(ICH):
                            nc.tensor.matmul(
                                out=o2[:],
                                lhsT=hj[:, k, ci * 128 : (ci + 1) * 128],
                                rhs=w2[:, k, n2 * 384 : (n2 + 1) * 384],
                                start=(k == 0), stop=(k == ICH - 1),
                            )
                        nc.vector.tensor_copy(
                            out=y[:, ci, n2 * 384 : (n2 + 1) * 384], in_=o2[:]
                        )
                ys.append(y)

            # ---------------- combine: out = shared + sum_j Pw_j^T y_j ------
            for c in range(TCH):
                arow = workp.tile([128, H], BF16, tag="arow")
                for n2 in range(2):
                    ps = psp.tile([128, 384], F32, tag="ps")
                    nc.tensor.matmul(
                        out=ps[:],
                        lhsT=hsh[:, c * 128 : (c + 1) * 128],
                        rhs=wsd[:, n2 * 384 : (n2 + 1) * 384],
                        start=True, stop=False,
                    )
                    for j in range(EPC):
                        for ci in range(2):
                            nc.tensor.matmul(
                                out=ps[:],
                                lhsT=pws[j][:, ci, c * 128 : (c + 1) * 128],
                                rhs=ys[j][:, ci, n2 * 384 : (n2 + 1) * 384],
                                start=False,
                                stop=(j == EPC - 1 and ci == 1),
                            )
                    nc.vector.tensor_copy(
                        out=arow[:, n2 * 384 : (n2 + 1) * 384], in_=ps[:]
                    )
                nc.sync.dma_start(
                    out=acc_d[c * 128 : (c + 1) * 128, :], in_=arow[:]
                )

            # ---------------- cross-core reduce (bf16) ----------------
            nc.gpsimd.collective_compute(
                "ReduceScatter",
                mybir.AluOpType.add,
                replica_groups=[list(range(NCORES))],
                ins=[acc_d[:, :]],
                outs=[rs_d[:, :]],
            )
            nc.sync.dma_start(out=out_d[:, :], in_=rs_d[:, :])

    return nc


def _pack_kT(a, dtype):
    """[H, N] -> [128, HCH*N] so each partition line is contiguous in DRAM.

    Element (p, k*N + t) = a[k*128 + p, t].
    """
    Hh, N = a.shape
    kch = Hh // 128
    return np.ascontiguousarray(
        a.reshape(kch, 128, N).transpose(1, 0, 2).reshape(128, kch * N)
    ).astype(dtype)


def make_core_inputs(inputs):
    """Host-side sharding: returns the per-core input maps (list of dicts)."""
    x = np.asarray(inputs["hidden_states"], np.float32)
    gate_w = np.asarray(inputs["gate_w"], np.float32)
    e_bias = np.asarray(inputs["e_bias"], np.float32)
    w1 = np.asarray(inputs["w1"], np.float32)
    w3 = np.asarray(inputs["w3"], np.float32)
    w2 = np.asarray(inputs["w2"], np.float32)
    ws_gate = np.asarray(inputs["ws_gate"], np.float32)
    ws_up = np.asarray(inputs["ws_up"], np.float32)
    ws_down = np.asarray(inputs["ws_down"], np.float32)

    xT = np.ascontiguousarray(x.T)                      # [H, T]
    xTp = _pack_kT(xT, np.float32)
    xTbp = _pack_kT(xT, NPBF16)
    x_bf = np.zeros((T + 1, H), NPBF16)
    x_bf[:T] = x.astype(NPBF16)
    tokid = (
        np.arange(128, dtype=np.float32)[:, None]
        + 128.0 * np.arange(TCH, dtype=np.float32)[None, :]
        + 1.0
    )  # (p, c) -> c*128 + p + 1
    slotc = (
        np.arange(16, dtype=np.float32)[:, None]
        + 16.0 * np.arange(16, dtype=np.float32)[None, :]
    )  # slot(p, f) = f*16 + p
    iotab = np.broadcast_to(
        np.arange(T, dtype=np.float32)[None, :], (128, T)
    ).copy()

    maps = []
    for r in range(NCORES):
        rot = np.roll(np.arange(E), -EPC * r)
        mine = rot[:EPC]
        w13p = np.empty((EPC, 128, HCH * 2 * I), NPBF16)
        w2p = np.empty((EPC, 128, ICH * H), NPBF16)
        for jj, e in enumerate(mine):
            w13T = np.concatenate([w1[e].T, w3[e].T], axis=1)  # [H, 2I]
            w13p[jj] = _pack_kT(w13T, NPBF16)
            w2p[jj] = _pack_kT(np.ascontiguousarray(w2[e].T), NPBF16)
        sl = slice(r * SIPC, (r + 1) * SIPC)
        maps.append(
            {
                "xTp": xTp,
                "xTbp": xTbp,
                "x_bf": x_bf,
                "gwp": _pack_kT(np.ascontiguousarray(gate_w[rot].T), np.float32),
                "ebias_b": np.broadcast_to(e_bias[rot], (128, E)).copy(),
                "w13p": w13p,
                "w2p": w2p,
                "wsgp": _pack_kT(np.ascontiguousarray(ws_gate[sl].T), NPBF16),
                "wsup": _pack_kT(np.ascontiguousarray(ws_up[sl].T), NPBF16),
                "wsdp": np.ascontiguousarray(ws_down[:, sl].T).astype(NPBF16),
                "tokid": tokid,
                "slotc": slotc,
                "iotab": iotab,
            }
        )
    return maps


_NC_CACHE = None


def kernel(**inputs) -> np.ndarray:
    global _NC_CACHE
    if _NC_CACHE is None:
        nc = build_nc()
        nc.finalize()
        _NC_CACHE = nc
    nc = _NC_CACHE
    in_maps = make_core_inputs(inputs)
    res = run_bass_kernel_spmd(nc, in_maps, list(range(NCORES)))
    out = np.concatenate([res.results[i]["out"] for i in range(NCORES)], axis=0)
    return out.astype(np.float32)


# revision 35
# speedup vs baseline: 1.5271x; 1.0717x over previous
"""DeepseekV3-style MoE block on 8 Trainium2 NeuronCores (expert-parallel).

Sharding strategy (v3, bf16 + matmul combine):
  - 64 routed experts sharded 8-per-core; expert columns rotated per core so
    each core's experts sit at columns 0..7 of its router output.
  - Router replicated, computed in f32r (selection must match reference).
  - Shared expert TP-sharded on intermediate dim (96 rows/core), bf16.
  - FFN weights/activations bf16; PSUM accumulation f32. Weights host-packed
    so every SBUF partition line is one contiguous DRAM read.
  - Token dispatch: dense combine weights -> per-expert compacted token lists
    AND compacted combine weights via parallel gpsimd sparse_gathers ->
    dma_gather(transpose=True) pulls x rows straight into [H-part, slot]
    layout (pad slots read the zero row T).
  - Token combine: NO scatter-add. Per expert a one-hot matrix
    Pw_j[slot, tok] = w_j[slot] * (tok == token(slot)) is built on the DVE;
    out[tok, :] = shared + sum_j Pw_j^T @ y_j accumulates in PSUM, 17
    matmuls per output chunk. Pad slots have token id T so they never match.
  - ReduceScatter(add) in bf16; host concats the 8 shards and upcasts.
"""

import numpy as np
import ml_dtypes

import concourse.bass as bass
import concourse.bacc as bacc
import concourse.mybir as mybir
import concourse.tile as tile
from concourse.bass_utils import run_bass_kernel_spmd
from concourse.masks import make_identity

F32 = mybir.dt.float32
F32R = mybir.dt.float32r
BF16 = mybir.dt.bfloat16
I16 = mybir.dt.int16
U32 = mybir.dt.uint32
U8 = mybir.dt.uint8

NPBF16 = ml_dtypes.bfloat16

# Model constants (hardcoded per contest rules)
E = 64          # experts
TOPK = 8
NG = 8          # groups
TOPKG = 4       # groups selected
SCALE = 2.5
H = 768         # hidden
I = 384         # routed expert intermediate
SI = 768        # shared expert intermediate
T = 1024        # tokens
NCORES = 8
EPC = E // NCORES     # experts per core = 8
SIPC = SI // NCORES   # shared-intermediate rows per core = 96
C = 256               # per-expert token capacity (max observed load is 224)
TCH = T // 128        # token chunks = 8
HCH = H // 128        # hidden chunks = 6
ICH = I // 128        # intermediate chunks = 3
BIG = 1.0e30


def build_nc():
    nc = bacc.Bacc(num_devices=NCORES)

    # ---------------- I/O (all host-packed; see make_core_inputs) ----------
    xTp_d = nc.declare_dram_parameter("xTp", [128, HCH * T], F32R, isOutput=False)
    gwp_d = nc.declare_dram_parameter("gwp", [128, HCH * E], F32R, isOutput=False)
    eb_d = nc.declare_dram_parameter("ebias_b", [128, E], F32, isOutput=False)
    tok_d = nc.declare_dram_parameter("tokid", [128, TCH], F32, isOutput=False)
    slot_d = nc.declare_dram_parameter("slotc", [16, 16], F32, isOutput=False)
    iota_d = nc.declare_dram_parameter("iotab", [128, T], F32, isOutput=False)
    xbf_d = nc.declare_dram_parameter("x_bf", [T + 1, H], BF16, isOutput=False)
    xTbp_d = nc.declare_dram_parameter("xTbp", [128, HCH * T], BF16, isOutput=False)
    w13_d = nc.declare_dram_parameter("w13p", [EPC, 128, HCH * 2 * I], BF16, isOutput=False)
    w2_d = nc.declare_dram_parameter("w2p", [EPC, 128, ICH * H], BF16, isOutput=False)
    wsg_d = nc.declare_dram_parameter("wsgp", [128, HCH * SIPC], BF16, isOutput=False)
    wsu_d = nc.declare_dram_parameter("wsup", [128, HCH * SIPC], BF16, isOutput=False)
    wsd_d = nc.declare_dram_parameter("wsdp", [SIPC, H], BF16, isOutput=False)
    out_d = nc.declare_dram_parameter("out", [T // NCORES, H], BF16, isOutput=True)

    # ---------------- internal DRAM ----------------
    vals_d = nc.dram_tensor("vals_d", [16, T], F32)    # rows 0..7 tokids, 8..15 weights
    wv_d = nc.dram_tensor("wv_d", [128, 2 * EPC], F32)   # per-slot weights, scrambled
    iv_d = nc.dram_tensor("iv_d", [128, 2 * EPC], F32)   # per-slot token ids, scrambled
    idx_d = nc.dram_tensor("idx_d", [16, EPC * 16], I16)
    acc0_d = nc.dram_tensor("acc0_d", [T // 2, H], BF16)
    acc1_d = nc.dram_tensor("acc1_d", [T // 2, H], BF16)
    rs0_d = nc.dram_tensor("rs0_d", [T // 2 // NCORES, H], BF16)
    rs1_d = nc.dram_tensor("rs1_d", [T // 2 // NCORES, H], BF16)

    with tile.TileContext(nc) as tc:
        with (
            tc.tile_pool(name="const", bufs=1) as constp,
            tc.tile_pool(name="xstream", bufs=2) as xsp,
            tc.tile_pool(name="wts", bufs=2) as wtsp,
            tc.tile_pool(name="route", bufs=1) as routep,
            tc.tile_pool(name="keep", bufs=1) as keepp,
            tc.tile_pool(name="small", bufs=2) as smallp,
            tc.tile_pool(name="work", bufs=2) as workp,
            tc.tile_pool(name="psum", bufs=8, space="PSUM") as psp,
        ):
            # ---------------- constants / inputs ----------------
            ident = constp.tile([128, 128], F32, tag="ident")
            make_identity(nc, ident[:])
            ebias = constp.tile([128, E], F32, tag="ebias")
            nc.sync.dma_start(out=ebias[:], in_=eb_d[:, :])
            tokid = constp.tile([128, TCH], F32, tag="tokid")
            nc.sync.dma_start(out=tokid[:], in_=tok_d[:, :])
            slotc = constp.tile([16, 16], F32, tag="slotc")
            nc.sync.dma_start(out=slotc[:], in_=slot_d[:, :])
            iotab = constp.tile([128, T], F32, tag="iotab")
            nc.sync.dma_start(out=iotab[:], in_=iota_d[:, :])

            gw = constp.tile([128, HCH, E], F32R, tag="gw")
            nc.sync.dma_start(out=gw[:], in_=gwp_d[:, :])
            xTb = constp.tile([128, HCH, T], BF16, tag="xTb")
            nc.sync.dma_start(out=xTb[:], in_=xTbp_d[:, :])
            wsg = constp.tile([128, HCH, SIPC], BF16, tag="wsg")
            nc.sync.dma_start(out=wsg[:], in_=wsg_d[:, :])
            wsu = constp.tile([128, HCH, SIPC], BF16, tag="wsu")
            nc.sync.dma_start(out=wsu[:], in_=wsu_d[:, :])
            wsd = constp.tile([SIPC, H], BF16, tag="wsd")
            nc.sync.dma_start(out=wsd[:], in_=wsd_d[:, :])

            # ---------------- router logits: logitsT = gw.T @ xT ------------
            lgsb = routep.tile([64, T], F32, tag="lgsb")
            lgp0 = psp.tile([64, 512], F32, tag="ps")
            lgp1 = psp.tile([64, 512], F32, tag="ps")
            lgps = [lgp0, lgp1]
            for k in range(HCH):
                xck = xsp.tile([128, T], F32R, tag="xck")
                nc.sync.dma_start(out=xck[:], in_=xTp_d[:, k * T : (k + 1) * T])
                for n in range(2):
                    nc.tensor.matmul(
                        out=lgps[n][:],
                        lhsT=gw[:, k, :],
                        rhs=xck[:, n * 512 : (n + 1) * 512],
                        start=(k == 0),
                        stop=(k == HCH - 1),
                    )
            for n in range(2):
                nc.vector.tensor_copy(
                    out=lgsb[:, n * 512 : (n + 1) * 512], in_=lgps[n][:]
                )

            # ---------------- routing (batched DVE over all chunks) ---------
            scores = routep.tile([128, TCH, E], F32, tag="scores")
            for c in range(TCH):
                lt = psp.tile([128, 64], F32, tag="ps")
                nc.tensor.transpose(
                    out=lt[:], in_=lgsb[:, c * 128 : (c + 1) * 128],
                    identity=ident[:64, :64],
                )
                nc.scalar.activation(
                    out=scores[:, c, :], in_=lt[:],
                    func=mybir.ActivationFunctionType.Sigmoid,
                )

            swb = routep.tile([128, TCH, E], F32, tag="swb")
            nc.vector.tensor_tensor(
                out=swb[:], in0=scores[:],
                in1=ebias[:, None, :].to_broadcast([128, TCH, E]),
                op=mybir.AluOpType.add,
            )
            swg = swb[:].rearrange("p c (g e) -> p (c g) e", e=NG)
            m1 = routep.tile([128, TCH * NG], F32, tag="m1")
            nc.vector.tensor_reduce(
                out=m1[:], in_=swg, axis=mybir.AxisListType.X,
                op=mybir.AluOpType.max,
            )
            eq = routep.tile([128, TCH * NG, NG], F32, tag="eq")
            nc.vector.tensor_tensor(
                out=eq[:], in0=swg,
                in1=m1[:, :, None].to_broadcast([128, TCH * NG, NG]),
                op=mybir.AluOpType.is_equal,
            )
            nc.vector.tensor_scalar(
                out=eq[:], in0=eq[:], scalar1=-BIG, scalar2=None,
                op0=mybir.AluOpType.mult,
            )
            nc.vector.tensor_add(out=eq[:], in0=eq[:], in1=swg)
            m2 = routep.tile([128, TCH * NG], F32, tag="m2")
            nc.vector.tensor_reduce(
                out=m2[:], in_=eq[:], axis=mybir.AxisListType.X,
                op=mybir.AluOpType.max,
            )
            gs = routep.tile([128, TCH, NG], F32, tag="gs")
            nc.vector.tensor_add(
                out=gs[:].rearrange("p c g -> p (c g)"), in0=m1[:], in1=m2[:]
            )
            g4s = routep.tile([128, TCH, 8], F32, tag="g4s")
            for c in range(TCH):
                nc.vector.max(out=g4s[:, c, :], in_=gs[:, c, :])
            gmask = routep.tile([128, TCH, NG], F32, tag="gmask")
            nc.vector.tensor_tensor(
                out=gmask[:], in0=gs[:],
                in1=g4s[:, :, TOPKG - 1 : TOPKG].to_broadcast([128, TCH, NG]),
                op=mybir.AluOpType.is_ge,
            )
            masked = routep.tile([128, TCH, E], F32, tag="masked")
            nc.vector.tensor_tensor(
                out=masked[:].rearrange("p c (g e) -> p (c g) e", e=NG),
                in0=swg,
                in1=gmask[:].rearrange("p c g -> p (c g)")[:, :, None]
                .to_broadcast([128, TCH * NG, NG]),
                op=mybir.AluOpType.mult,
            )
            t8s = routep.tile([128, TCH, 8], F32, tag="t8s")
            for c in range(TCH):
                nc.vector.max(out=t8s[:, c, :], in_=masked[:, c, :])
            nmask = routep.tile([128, TCH, E], F32, tag="nmask")
            nc.vector.tensor_tensor(
                out=nmask[:], in0=masked[:],
                in1=t8s[:, :, TOPK - 1 : TOPK].to_broadcast([128, TCH, E]),
                op=mybir.AluOpType.is_ge,
            )
            sel = routep.tile([128, TCH, E], F32, tag="sel")
            nc.vector.tensor_tensor(
                out=sel[:], in0=scores[:], in1=nmask[:], op=mybir.AluOpType.mult
            )
            den = routep.tile([128, TCH], F32, tag="den")
            nc.vector.tensor_reduce(
                out=den[:], in_=sel[:], axis=mybir.AxisListType.X,
                op=mybir.AluOpType.add,
            )
            nc.vector.tensor_scalar(
                out=den[:], in0=den[:], scalar1=1e-20, scalar2=None,
                op0=mybir.AluOpType.add,
            )
            rec = routep.tile([128, TCH], F32, tag="rec")
            nc.vector.reciprocal(out=rec[:], in_=den[:])
            nc.vector.tensor_scalar(
                out=rec[:], in0=rec[:], scalar1=SCALE, scalar2=None,
                op0=mybir.AluOpType.mult,
            )
            nc.vector.tensor_tensor(
                out=sel[:], in0=sel[:],
                in1=rec[:, :, None].to_broadcast([128, TCH, E]),
                op=mybir.AluOpType.mult,
            )

            # my experts' (cols 0..7) selection mask + compaction values
            mask8 = routep.tile([128, TCH, EPC], F32, tag="mask8")
            nc.vector.tensor_scalar(
                out=mask8[:], in0=sel[:, :, 0:EPC], scalar1=0.0, scalar2=None,
                op0=mybir.AluOpType.is_gt,
            )
            vw = routep.tile([128, TCH, 2 * EPC], F32, tag="vw")
            nc.vector.tensor_tensor(
                out=vw[:, :, 0:EPC], in0=mask8[:],
                in1=tokid[:, :, None].to_broadcast([128, TCH, EPC]),
                op=mybir.AluOpType.mult,
            )
            nc.vector.tensor_scalar(
                out=vw[:, :, 0:EPC], in0=vw[:, :, 0:EPC], scalar1=-1.0,
                scalar2=None, op0=mybir.AluOpType.add,
            )
            nc.vector.tensor_scalar(
                out=vw[:, :, EPC:], in0=sel[:, :, 0:EPC], scalar1=1.0,
                scalar2=None, op0=mybir.AluOpType.add,
            )
            nc.vector.tensor_tensor(
                out=vw[:, :, EPC:], in0=vw[:, :, EPC:], in1=mask8[:],
                op=mybir.AluOpType.mult,
            )
            nc.vector.tensor_scalar(
                out=vw[:, :, EPC:], in0=vw[:, :, EPC:], scalar1=-1.0,
                scalar2=None, op0=mybir.AluOpType.add,
            )

            valsT = routep.tile([16, T], F32, tag="valsT")
            for c in range(TCH):
                vt = psp.tile([16, 128], F32, tag="ps")
                nc.tensor.transpose(out=vt[:], in_=vw[:, c, :], identity=ident[:])
                nc.vector.tensor_copy(
                    out=valsT[:, c * 128 : (c + 1) * 128], in_=vt[:]
                )

            # valsT -> DRAM -> 16-partition-wrapped view (wrap t = p*64 + f
            # keeps partition lines contiguous; wrap order is irrelevant to
            # the compaction)
            nc.sync.dma_start(out=vals_d[:, :], in_=valsT[:])
            v16all = routep.tile([16, 2 * EPC, T // 16], F32, tag="v16all")
            nc.sync.dma_start(
                out=v16all[:],
                in_=bass.AP(vals_d, 0, [[T // 16, 16], [T, 2 * EPC], [1, T // 16]]),
            )

            # compact per-expert token lists + weights (gpsimd sparse_gather)
            idxf = routep.tile([16, EPC * 16], F32, tag="idxf")
            wvs = routep.tile([16, EPC, 16], F32, tag="wvs")
            nfound = routep.tile([1, 2 * EPC], U32, tag="nfound")
            nc.vector.memset(idxf[:], -1.0)
            nc.vector.memset(wvs[:], 0.0)
            for j in range(EPC):
                nc.gpsimd.sparse_gather(
                    out=idxf[:, j * 16 : (j + 1) * 16],
                    in_=v16all[:, j, :],
                    num_found=nfound[:, j : j + 1],
                )
                nc.gpsimd.sparse_gather(
                    out=wvs[:, j, :],
                    in_=v16all[:, EPC + j, :],
                    num_found=nfound[:, EPC + j : EPC + j + 1],
                )
            # sanitize: sparse_gather writes ARBITRARY (possibly NaN) values
            # beyond num_found on hardware, so pads must be replaced via
            # select() (NaN-garbage-proof) using slot < count masks.
            cntf = routep.tile([1, 2 * EPC], F32, tag="cntf")
            nc.vector.tensor_copy(out=cntf[:], in_=nfound[:])
            cntb = routep.tile([16, 2 * EPC], F32, tag="cntb")
            nc.gpsimd.partition_broadcast(out_ap=cntb[:], in_ap=cntf[:])
            padT = routep.tile([16, 16], F32, tag="padT")
            nc.vector.memset(padT[:], float(T))
            zero16 = routep.tile([16, 16], F32, tag="zero16")
            nc.vector.memset(zero16[:], 0.0)
            idxt = routep.tile([16, EPC * 16], F32, tag="idxt")
            wvc = routep.tile([16, EPC, 16], F32, tag="wvc")
            for j in range(EPC):
                keep = smallp.tile([16, 16], U8, tag="keep")
                nc.vector.tensor_scalar(
                    out=keep[:], in0=slotc[:], scalar1=cntb[:, j : j + 1],
                    scalar2=None, op0=mybir.AluOpType.is_lt,
                )
                nc.vector.select(
                    out=idxt[:, j * 16 : (j + 1) * 16], mask=keep[:],
                    on_true=idxf[:, j * 16 : (j + 1) * 16], on_false=padT[:],
                )
                nc.vector.select(
                    out=wvc[:, j, :], mask=keep[:],
                    on_true=wvs[:, j, :], on_false=zero16[:],
                )
            idxt16 = routep.tile([16, EPC * 16], I16, tag="idxt16")
            nc.vector.tensor_copy(out=idxt16[:], in_=idxt[:])

            # replicate idx rows to all 8 16-partition groups (DRAM bounce)
            nc.sync.dma_start(out=idx_d[:, :], in_=idxt16[:])
            idxr = routep.tile([128, EPC * 16], I16, tag="idxr")
            nc.sync.dma_start(
                out=idxr[:],
                in_=bass.AP(idx_d, 0, [[0, 8], [EPC * 16, 16], [1, EPC * 16]]),
            )
            # weights and token ids -> DRAM with scatter AP so the read back
            # is col[p, j*2+ci] = value at slot ci*128+p of expert j
            scr_ap = [[16, 16], [2, EPC], [1, 2], [256, 8]]
            nc.sync.dma_start(out=bass.AP(wv_d, 0, scr_ap), in_=wvc[:])
            nc.sync.dma_start(
                out=bass.AP(iv_d, 0, scr_ap),
                in_=idxt[:].rearrange("p (j f) -> p j f", f=16),
            )
            wcol = routep.tile([128, 2 * EPC], F32, tag="wcol")
            nc.sync.dma_start(out=wcol[:], in_=wv_d[:, :])
            icol = routep.tile([128, 2 * EPC], F32, tag="icol")
            nc.sync.dma_start(out=icol[:], in_=iv_d[:, :])

            # ---------------- shared expert stage 1 (TP slice, bf16) --------
            hsh = routep.tile([SIPC, T], BF16, tag="hsh")
            for n in range(2):
                hg = psp.tile([SIPC, 512], F32, tag="ps")
                hu = psp.tile([SIPC, 512], F32, tag="ps")
                for k in range(HCH):
                    nc.tensor.matmul(
                        out=hg[:], lhsT=wsg[:, k, :],
                        rhs=xTb[:, k, n * 512 : (n + 1) * 512],
                        start=(k == 0), stop=(k == HCH - 1),
                    )
                for k in range(HCH):
                    nc.tensor.matmul(
                        out=hu[:], lhsT=wsu[:, k, :],
                        rhs=xTb[:, k, n * 512 : (n + 1) * 512],
                        start=(k == 0), stop=(k == HCH - 1),
                    )
                hsig = smallp.tile([SIPC, 512], F32, tag="hsig")
                nc.scalar.activation(
                    out=hsig[:], in_=hg[:],
                    func=mybir.ActivationFunctionType.Sigmoid,
                )
                nc.vector.tensor_tensor(
                    out=hsig[:], in0=hsig[:], in1=hg[:], op=mybir.AluOpType.mult
                )
                nc.vector.tensor_tensor(
                    out=hsh[:, n * 512 : (n + 1) * 512], in0=hsig[:], in1=hu[:],
                    op=mybir.AluOpType.mult,
                )

            # ---------------- routed experts (bf16) ----------------
            ys = []
            for j in range(EPC):
                w13 = wtsp.tile([128, HCH, 2 * I], BF16, tag="w13")
                nc.sync.dma_start(out=w13[:], in_=w13_d[j])
                w2 = wtsp.tile([128, ICH, H], BF16, tag="w2")
                nc.sync.dma_start(out=w2[:], in_=w2_d[j])

                xgT = workp.tile([128, HCH, C], BF16, tag="xgT")
                nc.gpsimd.dma_gather(
                    out_ap=xgT[:], in_ap=xbf_d[:, :],
                    idxs_ap=idxr[:, j * 16 : (j + 1) * 16],
                    num_idxs=C, num_idxs_reg=C, elem_size=H,
                    transpose=True,
                )

                hj = workp.tile([128, ICH, C], BF16, tag="hj")
                for m in range(ICH):
                    h13 = psp.tile([128, 512], F32, tag="ps")
                    for k in range(HCH):
                        nc.tensor.matmul(
                            out=h13[:, 0:C],
                            lhsT=w13[:, k, m * 128 : (m + 1) * 128],
                            rhs=xgT[:, k, :],
                            start=(k == 0), stop=(k == HCH - 1),
                        )
                    for k in range(HCH):
                        nc.tensor.matmul(
                            out=h13[:, C : 2 * C],
                            lhsT=w13[:, k, I + m * 128 : I + (m + 1) * 128],
                            rhs=xgT[:, k, :],
                            start=(k == 0), stop=(k == HCH - 1),
                        )
                    hsil = workp.tile([128, C], F32, tag="hsil")
                    nc.scalar.activation(
                        out=hsil[:], in_=h13[:, 0:C],
                        func=mybir.ActivationFunctionType.Sigmoid,
                    )
                    nc.vector.tensor_tensor(
                        out=hsil[:], in0=hsil[:], in1=h13[:, 0:C],
                        op=mybir.AluOpType.mult,
                    )
                    nc.vector.tensor_tensor(
                        out=hj[:, m, :], in0=hsil[:], in1=h13[:, C : 2 * C],
                        op=mybir.AluOpType.mult,
                    )

                y = keepp.tile([128, C // 128, H], BF16, tag=f"y_{j}")
                for ci in range(C // 128):
                    for n2 in range(2):
                        o2 = psp.tile([128, 384], F32, tag="ps")
                        for k in range(ICH):
                            nc.tensor.matmul(
                                out=o2[:],
                                lhsT=hj[:, k, ci * 128 : (ci + 1) * 128],
                                rhs=w2[:, k, n2 * 384 : (n2 + 1) * 384],
                                start=(k == 0), stop=(k == ICH - 1),
                            )
                        nc.vector.tensor_copy(
                            out=y[:, ci, n2 * 384 : (n2 + 1) * 384], in_=o2[:]
                        )
                ys.append(y)

            # one-hot combine matrices: Pw_j[p, ci, t] =
            #   w_j[slot ci*128+p] * (t == token(slot ci*128+p))
            # (emitted after the FFN loop so the DVE prioritizes silu work)
            pws = []
            for j in range(EPC):
                pw = keepp.tile([128, 2, T], BF16, tag=f"pw_{j}")
                for ci in range(2):
                    nc.vector.tensor_scalar(
                        out=pw[:, ci, :], in0=iotab[:],
                        scalar1=icol[:, j * 2 + ci : j * 2 + ci + 1],
                        scalar2=None, op0=mybir.AluOpType.is_equal,
                    )
                    nc.vector.tensor_scalar(
                        out=pw[:, ci, :], in0=pw[:, ci, :],
                        scalar1=wcol[:, j * 2 + ci : j * 2 + ci + 1],
                        scalar2=None, op0=mybir.AluOpType.mult,
                    )
                pws.append(pw)

            # ---------------- combine: out = shared + sum_j Pw_j^T y_j ------
            # split in two token halves; the first half's ReduceScatter
            # overlaps the second half's matmuls
            accs = [acc0_d, acc1_d]
            for c in range(TCH):
                arow = workp.tile([128, H], BF16, tag="arow")
                for n2 in range(2):
                    ps = psp.tile([128, 384], F32, tag="ps")
                    nc.tensor.matmul(
                        out=ps[:],
                        lhsT=hsh[:, c * 128 : (c + 1) * 128],
                        rhs=wsd[:, n2 * 384 : (n2 + 1) * 384],
                        start=True, stop=False,
                    )
                    for j in range(EPC):
                        for ci in range(2):
                            nc.tensor.matmul(
                                out=ps[:],
                                lhsT=pws[j][:, ci, c * 128 : (c + 1) * 128],
                                rhs=ys[j][:, ci, n2 * 384 : (n2 + 1) * 384],
                                start=False,
                                stop=(j == EPC - 1 and ci == 1),
                            )
                    nc.vector.tensor_copy(
                        out=arow[:, n2 * 384 : (n2 + 1) * 384], in_=ps[:]
                    )
                half, crel = divmod(c, TCH // 2)
                nc.sync.dma_start(
                    out=accs[half][crel * 128 : (crel + 1) * 128, :], in_=arow[:]
                )
                if c == TCH // 2 - 1:
                    nc.gpsimd.collective_compute(
                        "ReduceScatter",
                        mybir.AluOpType.add,
                        replica_groups=[list(range(NCORES))],
                        ins=[acc0_d[:, :]],
                        outs=[rs0_d[:, :]],
                    )

            # ---------------- cross-core reduce, second half (bf16) ---------
            nc.gpsimd.collective_compute(
                "ReduceScatter",
                mybir.AluOpType.add,
                replica_groups=[list(range(NCORES))],
                ins=[acc1_d[:, :]],
                outs=[rs1_d[:, :]],
            )
            nc.sync.dma_start(out=out_d[0 : T // 2 // NCORES, :], in_=rs0_d[:, :])
            nc.sync.dma_start(out=out_d[T // 2 // NCORES :, :], in_=rs1_d[:, :])

    return nc


def _pack_kT(a, dtype):
    """[H, N] -> [128, HCH*N] so each partition line is contiguous in DRAM.

    Element (p, k*N + t) = a[k*128 + p, t].
    """
    Hh, N = a.shape
    kch = Hh // 128
    return np.ascontiguousarray(
        a.reshape(kch, 128, N).transpose(1, 0, 2).reshape(128, kch * N)
    ).astype(dtype)


def make_core_inputs(inputs):
    """Host-side sharding: returns the per-core input maps (list of dicts)."""
    x = np.asarray(inputs["hidden_states"], np.float32)
    gate_w = np.asarray(inputs["gate_w"], np.float32)
    e_bias = np.asarray(inputs["e_bias"], np.float32)
    w1 = np.asarray(inputs["w1"], np.float32)
    w3 = np.asarray(inputs["w3"], np.float32)
    w2 = np.asarray(inputs["w2"], np.float32)
    ws_gate = np.asarray(inputs["ws_gate"], np.float32)
    ws_up = np.asarray(inputs["ws_up"], np.float32)
    ws_down = np.asarray(inputs["ws_down"], np.float32)

    xT = np.ascontiguousarray(x.T)                      # [H, T]
    xTp = _pack_kT(xT, np.float32)
    xTbp = _pack_kT(xT, NPBF16)
    x_bf = np.zeros((T + 1, H), NPBF16)
    x_bf[:T] = x.astype(NPBF16)
    tokid = (
        np.arange(128, dtype=np.float32)[:, None]
        + 128.0 * np.arange(TCH, dtype=np.float32)[None, :]
        + 1.0
    )  # (p, c) -> c*128 + p + 1
    slotc = (
        np.arange(16, dtype=np.float32)[:, None]
        + 16.0 * np.arange(16, dtype=np.float32)[None, :]
    )  # slot(p, f) = f*16 + p
    iotab = np.broadcast_to(
        np.arange(T, dtype=np.float32)[None, :], (128, T)
    ).copy()

    maps = []
    for r in range(NCORES):
        rot = np.roll(np.arange(E), -EPC * r)
        mine = rot[:EPC]
        w13p = np.empty((EPC, 128, HCH * 2 * I), NPBF16)
        w2p = np.empty((EPC, 128, ICH * H), NPBF16)
        for jj, e in enumerate(mine):
            w13T = np.concatenate([w1[e].T, w3[e].T], axis=1)  # [H, 2I]
            w13p[jj] = _pack_kT(w13T, NPBF16)
            w2p[jj] = _pack_kT(np.ascontiguousarray(w2[e].T), NPBF16)
        sl = slice(r * SIPC, (r + 1) * SIPC)
        maps.append(
            {
                "xTp": xTp,
                "xTbp": xTbp,
                "x_bf": x_bf,
                "gwp": _pack_kT(np.ascontiguousarray(gate_w[rot].T), np.float32),
                "ebias_b": np.broadcast_to(e_bias[rot], (128, E)).copy(),
                "w13p": w13p,
                "w2p": w2p,
                "wsgp": _pack_kT(np.ascontiguousarray(ws_gate[sl].T), NPBF16),
                "wsup": _pack_kT(np.ascontiguousarray(ws_up[sl].T), NPBF16),
                "wsdp": np.ascontiguousarray(ws_down[:, sl].T).astype(NPBF16),
                "tokid": tokid,
                "slotc": slotc,
                "iotab": iotab,
            }
        )
    return maps


_NC_CACHE = None


def assemble(shards) -> np.ndarray:
    """Reassemble the full [T, H] output from the 8 per-core [128, H] shards.

    Core r's shard rows 0:64 are tokens [r*64, (r+1)*64) (first RS half) and
    rows 64:128 are tokens [T/2 + r*64, T/2 + (r+1)*64) (second RS half).
    """
    hh = T // 2 // NCORES  # 64
    out = np.empty((T, H), np.float32)
    for r, sh in enumerate(shards):
        sh = np.asarray(sh, np.float32)
        out[r * hh : (r + 1) * hh] = sh[0:hh]
        out[T // 2 + r * hh : T // 2 + (r + 1) * hh] = sh[hh:]
    return out


def kernel(**inputs) -> np.ndarray:
    global _NC_CACHE
    if _NC_CACHE is None:
        nc = build_nc()
        nc.finalize()
        _NC_CACHE = nc
    nc = _NC_CACHE
    in_maps = make_core_inputs(inputs)
    res = run_bass_kernel_spmd(nc, in_maps, list(range(NCORES)))
    return assemble([res.results[i]["out"] for i in range(NCORES)])
